# revision 34
# baseline (speedup 1.0000x reference)
"""Trainium2 Bass kernel for nn_BEE_Bin2Symbol (hyper-decoder + masked-conv
autoregressive MLP decoder).

Architecture (v2, latency-oriented):
- Sequential phase runs all GEMMs in [C_out-on-partitions, B-pixels-free]
  orientation (weights stationary as lhsT, activations moving): matmul cost
  scales with B<=16, transposes disappear, and each layer's nonlinearity is a
  single DVE scalar_tensor_tensor op  max(0.01*z, z)  reading PSUM directly.
- 140 slope-3 anti-diagonal wavefronts. Per step the critical chain is
  fresh-tap MMs -> lrelu(z0) -> W1 MMs -> lrelu(z1) -> ... -> z5 -> Y-add.
  Everything else (old-tap ctx GEMMs, f-part, bias seeds, phase-P conv
  streaming) is issued into the PE gaps between the chain's engine hops.
- Fresh taps (age-1) use precomposed G = W0c @ Wd_tap so they feed z0
  directly; old taps accumulate a ctx PSUM a step ahead, evicted by the
  scalar engine (ACT) off the critical path.
- All sequential-phase weights/activations in bf16 (PSUM accumulate f32).
- Hyper-decoder (2 stride-2 deconvs + 3x3 conv) in f32r as phase-decomposed
  GEMMs; conv2's last 3 row-blocks stream into the first ~15 wavefront steps.
"""
import sys, os
sys.path.insert(0, "/opt/trn_rl_repo")

import numpy as np

import concourse.bass as bass
import concourse.bacc as bacc
import concourse.mybir as mybir
import concourse.tile as tile
from concourse.masks import make_identity

F32 = mybir.dt.float32
F32R = mybir.dt.float32r
BF16 = mybir.dt.bfloat16

H, W = 32, 48
HP, WP = H + 4, W + 4            # padded image 36 x 52
NPIX = H * W
NSTEPS = 3 * (H - 1) + (W - 1) + 1   # 140
DIAG = WP - 3                    # 49: wavefront-diagonal stride in padded img

# taps (dy, dx): source pixel = (i-dy, j-dx); ctx_w index (ky,kx) = (2-dy, 2-dx)
TAPS = [(2, 2), (2, 1), (2, 0), (2, -1), (2, -2),
        (1, 2), (1, 1), (1, 0), (1, -1), (1, -2),
        (0, 1), (0, 2)]
FRESH_TAPS = [(1, -2), (0, 1)]                # age-1 taps (need step t-1)
OLD_TAPS = [d for d in TAPS if d not in FRESH_TAPS]

DIMS = [768, 640, 512, 384, 320, 256, 192]   # MLP dims; layer l: DIMS[l]->DIMS[l+1]
# z1..z5 chunk offsets inside the shared zs psum tile [128, 14, 16]
ZOFF = {1: 0, 2: 4, 3: 7, 4: 10, 5: 12}


def cdiv(a, b):
    return (a + b - 1) // b


def chunks_of(n, c=128):
    return [(s, min(c, n - s)) for s in range(0, n, c)]


def _ap(tile_ap, slot_off, elem_off, plist):
    """Build a custom AP into a [128, S, F]-shaped sbuf/psum tile."""
    base = tile_ap[:]
    return bass.AP(base.tensor, base.offset + slot_off + elem_off, plist)


def step_geom(t):
    i_lo = max(0, cdiv(t - (W - 1), 3))
    i_hi = min(H - 1, t // 3)
    return i_lo, i_hi - i_lo + 1, t - 3 * i_lo


def build(nsteps=NSTEPS):
    nc = bacc.Bacc()

    # ---------------- DRAM I/O ----------------
    di = {}
    di['z_hat'] = nc.dram_tensor('z_hat', [1, 192, 8, 12], F32, kind="ExternalInput")
    di['w_hat'] = nc.dram_tensor('w_hat', [1, 192, 32, 48], F32, kind="ExternalInput")
    di['hs_dw0'] = nc.dram_tensor('hs_dw0', [192, 192, 5, 5], F32, kind="ExternalInput")
    di['hs_db0'] = nc.dram_tensor('hs_db0', [192], F32, kind="ExternalInput")
    di['hs_dw1'] = nc.dram_tensor('hs_dw1', [192, 288, 5, 5], F32, kind="ExternalInput")
    di['hs_db1'] = nc.dram_tensor('hs_db1', [288], F32, kind="ExternalInput")
    di['hs_cw2'] = nc.dram_tensor('hs_cw2', [384, 288, 3, 3], F32, kind="ExternalInput")
    di['hs_cb2'] = nc.dram_tensor('hs_cb2', [384], F32, kind="ExternalInput")
    di['ctx_w'] = nc.dram_tensor('ctx_w', [384, 192, 5, 5], F32, kind="ExternalInput")
    di['ctx_b'] = nc.dram_tensor('ctx_b', [384], F32, kind="ExternalInput")
    for li in range(6):
        di[f'ep_w{li}'] = nc.dram_tensor(f'ep_w{li}', [DIMS[li + 1], DIMS[li]], F32,
                                         kind="ExternalInput")
        di[f'ep_b{li}'] = nc.dram_tensor(f'ep_b{li}', [DIMS[li + 1]], F32,
                                         kind="ExternalInput")
    out = nc.dram_tensor('out', [1, 192, 32, 48], F32, kind="ExternalOutput")

    with tile.TileContext(nc) as tc:
        with tc.tile_pool(name="pp", bufs=1) as pp, \
             tc.tile_pool(name="pps", bufs=1, space="PSUM") as pps:

            ident = pp.tile([128, 128], F32)
            make_identity(nc, ident[:])

            # ---------- persistent state ----------
            Yimg = pp.tile([128, 2, HP * WP], BF16)   # decoded image (padded)
            nc.vector.memset(Yimg[:], 0.0)
            wimg = pp.tile([128, 2, NPIX], F32)       # w_hat residual (compact)
            fm1 = pp.tile([128, 3, NPIX], BF16)       # conv2 output [384, 1536]
            m2 = pp.tile([128, 3, 34 * 50], BF16)     # deconv1 out (padded 34x50)
            nc.gpsimd.memset(m2[:], 0.0)
            cw2T = pp.tile([128, 3, 3, 9 * 128], BF16)  # [cin, mi, si, k*128+o]

            # transposed weights (bf16)
            W0fT = pp.tile([128, 3, 640], BF16)
            W0cT = pp.tile([128, 3, 640], BF16)
            WT = {}
            for li in range(1, 6):
                WT[li] = pp.tile([128, cdiv(DIMS[li], 128), DIMS[li + 1]], BF16,
                                 tag=f"W{li}T", name=f"W{li}T")
            GT = [pp.tile([128, 2, 640], BF16, tag=f"GT{k}", name=f"GT{k}") for k in range(2)]
            # relu-decomposition composites (0.01*z linear flow)
            PT = {}   # PT[l] = 0.0099*(W_{l+1} W_l)^T  -> feeds z_{l+1} from r_{l-1}
            for li in range(1, 5):
                PT[li] = pp.tile([128, cdiv(DIMS[li], 128), DIMS[li + 2]], BF16,
                                 tag=f"PT{li}", name=f"PT{li}")
            F1T = pp.tile([128, 3, 512], BF16)   # 0.01*(W1 W0f)^T
            C1T = pp.tile([128, 3, 512], BF16)   # 0.01*(W1 W0c)^T
            G1T = [pp.tile([128, 2, 512], BF16, tag=f"G1T{k}", name=f"G1T{k}")
                   for k in range(2)]            # 0.01*(W1 G_tap)^T
            wbrow = [None] + [pp.tile([1, DIMS[li + 1]], BF16, tag=f"wb{li}", name=f"wb{li}")
                              for li in range(1, 6)]   # 0.01*W_l b_{l-1} rows
            identB = pp.tile([128, 128], BF16)
            make_identity(nc, identB[:])
            WdT = {}
            for d in OLD_TAPS:
                WdT[d] = pp.tile([128, 2, 384], BF16, tag=f"Wd{d[0]}_{d[1]}", name=f"Wd{d[0]}_{d[1]}")

            # bias rows (lhsT for K=1 seed matmuls)
            brow = [pp.tile([1, DIMS[li + 1]], BF16, tag=f"b{li}", name=f"b{li}") for li in range(6)]
            ctxb = pp.tile([1, 384], BF16)
            ones = pp.tile([1, 16], BF16)
            nc.vector.memset(ones[:], 1.0)

            # sequential-phase activations (single tiles; WAR handled by sems)
            xs = {li: pp.tile([128, cdiv(DIMS[li], 128), 16], BF16, tag=f"x{li}", name=f"x{li}")
                  for li in range(1, 6)}
            Xc = pp.tile([128, 3, 16], BF16)          # evicted old-ctx

            # persistent psum: z0+ctx ring (2 banks), mlp zs (1), conv stream (1)
            zc = [pps.tile([128, 8, 16], F32, tag=f"zc{s}", name=f"zc{s}") for s in range(2)]
            zs = pps.tile([128, 14, 16], F32, tag="zs")
            nc.vector.memset(zs[:], 0.0)
            nc.vector.memset(zc[0][:], 0.0)
            nc.vector.memset(zc[1][:], 0.0)

            # ============ PROLOGUE 1: hyper-decoder (DMA priority) ============
            with tc.tile_pool(name="proB", bufs=2) as pro, \
                 tc.tile_pool(name="prpsB", bufs=2, space="PSUM") as prps:

                # SP queue: deconv0 weights lead everything
                dw0t = []
                for mi, (ms, mw) in enumerate(chunks_of(192)):
                    dw = pro.tile([128, 2, 128 * 25], F32R, tag="dw", name="dw", bufs=2)
                    for ci, (cs, cww) in enumerate(chunks_of(192)):
                        nc.sync.dma_start(
                            dw[0:cww, ci, 0:mw * 25],
                            di['hs_dw0'].ap()[cs:cs + cww, ms:ms + mw]
                            .rearrange("c o kh kw -> c (o kh kw)").bitcast(F32R))
                    dw0t.append(dw)

                # Pool/SWDGE queue: small loads (zp first - deconv0 input)
                zp = pro.tile([128, 2, 10 * 14], F32R, tag="zp", bufs=1)
                nc.vector.memset(zp[:].bitcast(F32), 0.0)
                zv = di['z_hat'].ap()[0]
                for ci, (s, cw) in enumerate(chunks_of(192)):
                    dst = _ap(zp, ci * 140, 14 + 1, [[2 * 140, cw], [14, 8], [1, 12]])
                    nc.gpsimd.dma_start(dst, zv[s:s + cw].bitcast(F32R))

                def load_bias_col(name, n):
                    nch = cdiv(n, 128)
                    t = pp.tile([128, nch], F32, tag=f"b_{name}", name=f"b_{name}")
                    nc.vector.memset(t[:], 0.0)
                    for ci, (s, w_) in enumerate(chunks_of(n)):
                        nc.gpsimd.dma_start(t[0:w_, ci:ci + 1], di[name][s:s + w_][:, None])
                    return t
                b_d0 = load_bias_col('hs_db0', 192)
                b_d1 = load_bias_col('hs_db1', 288)
                b_c2 = load_bias_col('hs_cb2', 384)

                def load_brow(dst, dram, n):
                    st = pro.tile([1, 640], F32, tag="brs", name="brs", bufs=1)
                    nc.gpsimd.dma_start(st[0:1, 0:n], dram.ap()[None, :])
                    nc.vector.tensor_copy(dst[0:1, 0:n], st[0:1, 0:n])
                for li in range(6):
                    load_brow(brow[li], di[f'ep_b{li}'], DIMS[li + 1])
                load_brow(ctxb, di['ctx_b'], 384)

                whv = di['w_hat'].ap()[0]
                for ci, (s, cw) in enumerate(chunks_of(192)):
                    nc.gpsimd.dma_start(
                        wimg[0:cw, ci, :].rearrange("p (h w) -> p h w", h=H),
                        whv[s:s + cw])

                m1 = pro.tile([128, 2, 18 * 26], F32R, tag="m1", bufs=1)
                nc.vector.memset(m1[:].bitcast(F32), 0.0)

                def deconv_chunk(inp_t, inp_hw, w_t, cin, mw, mi, out_t, bias_t):
                    hi, wi = inp_hw
                    ip_w = wi + 2
                    op_w = 2 * wi + 2
                    for py in range(2):
                        for px in range(2):
                            ps = prps.tile([128, 16 * 24], F32, tag="dps")
                            first = True
                            taps = [(u, v) for u in range(py, 5, 2) for v in range(px, 5, 2)]
                            for ti, (u, v) in enumerate(taps):
                                dy = (py + 2 - u) // 2
                                dx = (px + 2 - v) // 2
                                for ci, (cs, cww) in enumerate(chunks_of(cin)):
                                    lhsT = _ap(w_t, ci * 128 * 25, u * 5 + v,
                                               [[2 * 128 * 25, cww], [25, mw]])
                                    rhs = _ap(inp_t, ci * (hi + 2) * ip_w,
                                              (1 + dy) * ip_w + (1 + dx),
                                              [[2 * (hi + 2) * ip_w, cww], [ip_w, hi], [1, wi]])
                                    last = (ti == len(taps) - 1) and (ci == len(chunks_of(cin)) - 1)
                                    nc.tensor.matmul(ps[0:mw, 0:hi * wi], lhsT, rhs,
                                                     start=first, stop=last,
                                                     skip_group_check=True)
                                    first = False
                            dst = _ap(out_t, mi * (2 * hi + 2) * op_w,
                                      (py + 1) * op_w + (px + 1),
                                      [[out_t.shape[1] * (2 * hi + 2) * op_w, mw],
                                       [2 * op_w, hi], [2, wi]])
                            nc.scalar.activation(
                                dst, ps[0:mw, 0:hi * wi].rearrange("p (a b) -> p a b", a=hi),
                                mybir.ActivationFunctionType.Lrelu,
                                bias=bias_t[0:mw, mi][:, None], alpha=0.01)

                # ACT queue: dw1 (so SP-queue dw rotation can't block it)
                dw1t = []
                for mi, (ms, mw) in enumerate(chunks_of(288)):
                    dw = pro.tile([128, 2, 128 * 25], F32R, tag="dw", name="dw1", bufs=2)
                    for ci, (cs, cww) in enumerate(chunks_of(192)):
                        nc.scalar.dma_start(
                            dw[0:cww, ci, 0:mw * 25],
                            di['hs_dw1'].ap()[cs:cs + cww, ms:ms + mw]
                            .rearrange("c o kh kw -> c (o kh kw)").bitcast(F32R))
                    dw1t.append(dw)

                # deconv0: z[192,8,12] -> m1[192,16,24]
                for mi, (ms, mw) in enumerate(chunks_of(192)):
                    deconv_chunk(zp, (8, 12), dw0t[mi], 192, mw, mi, m1, b_d0)

                # deconv1: m1[192,16,24] -> m2[288,32,48]
                for mi, (ms, mw) in enumerate(chunks_of(288)):
                    deconv_chunk(m1, (16, 24), dw1t[mi], 192, mw, mi, m2, b_d1)

            # ============ PROLOGUE 2: weight transposes ============
            with tc.tile_pool(name="pro", bufs=2) as pro, \
                 tc.tile_pool(name="prps", bufs=2, space="PSUM") as prps:

                tp_count = [0]
                def evict(dst_ap, src_ap, scale=None):
                    if scale is None and tp_count[0] % 2 == 0:
                        nc.vector.tensor_copy(dst_ap, src_ap)
                    elif scale is None:
                        nc.scalar.activation(dst_ap, src_ap,
                                             mybir.ActivationFunctionType.Copy)
                    elif tp_count[0] % 2 == 0:
                        nc.vector.tensor_scalar_mul(dst_ap, src_ap, float(scale))
                    else:
                        nc.scalar.activation(dst_ap, src_ap,
                                             mybir.ActivationFunctionType.Copy,
                                             scale=float(scale))
                    tp_count[0] += 1

                # SP queue (free after dw0): ep_w0 then ctx_w
                def wnat0_half(h):
                    t = pro.tile([128, 3, 768], F32, tag="wnat0", name="wnat0", bufs=1)
                    for mi, (ms, mw) in enumerate(chunks_of(640)):
                        if mi // 3 != h:
                            continue
                        nc.sync.dma_start(t[0:mw, mi % 3, 0:768],
                                          di['ep_w0'].ap()[ms:ms + mw])
                    return t


                # ep_w0 -> W0fT / W0cT, batched evicts, two wnat0 halves
                W0fN = pro.tile([128, 5, 384], BF16, tag="w0fn", bufs=1)
                W0cN = pro.tile([128, 5, 384], BF16, tag="w0cn", bufs=1)
                for h in range(2):
                    wnat0 = wnat0_half(h)
                    mchunks = list(enumerate(chunks_of(640)))[h * 3:(h + 1) * 3]
                    for mi, (ms, mw) in mchunks:
                        evict(W0fN[0:mw, mi, 0:384], wnat0[0:mw, mi % 3, 0:384])
                        evict(W0cN[0:mw, mi, 0:384], wnat0[0:mw, mi % 3, 384:768])
                    for ci in range(6):
                        cww = 128
                        pt = prps.tile([128, 4, 128], F32, tag="tp")
                        for k, (mi, (ms, mw)) in enumerate(mchunks):
                            nc.tensor.transpose(pt[0:cww, k, 0:mw],
                                                wnat0[0:mw, mi % 3, ci * 128:ci * 128 + cww],
                                                ident[0:mw, 0:mw])
                        ms0 = mchunks[0][1][0]
                        tw = sum(mw for _, (ms, mw) in mchunks)
                        span = pt[:, 0:len(mchunks), :].rearrange("p a b -> p (a b)")[0:cww, 0:tw]
                        if ci < 3:
                            evict(W0fT[0:cww, ci, ms0:ms0 + tw], span)
                        else:
                            evict(W0cT[0:cww, ci - 3, ms0:ms0 + tw], span)

                # ctx_w -> WdT (old taps) + WdNf (fresh, bf16), cwn mi-outer
                WdNf = [pro.tile([128, 3, 192], BF16, tag=f"WdNf{f}", name=f"WdNf{f}", bufs=1)
                        for f in range(2)]
                for mi in range(3):
                    cwn = pro.tile([128, 192 * 25], F32R, tag="cwn", name="cwn", bufs=1)
                    nc.sync.dma_start(
                        cwn[:],
                        di['ctx_w'].ap()[mi * 128:(mi + 1) * 128]
                        .rearrange("o c kh kw -> o (c kh kw)").bitcast(F32R))
                    for d in OLD_TAPS:
                        ky, kx = 2 - d[0], 2 - d[1]
                        pt = prps.tile([128, 4, 128], F32, tag="tp")
                        for ci, (cs, cww) in enumerate(chunks_of(192)):
                            sap = _ap(cwn, 0, cs * 25 + ky * 5 + kx,
                                      [[192 * 25, 128], [25, cww]]).bitcast(F32)
                            nc.tensor.transpose(pt[0:cww, ci, 0:128], sap, ident[:])
                        dst = _ap(WdT[d], 0, mi * 128,
                                  [[2 * 384, 128], [384, 2], [1, 128]])
                        evict(dst, pt[:, 0:2, 0:128])
                    for f, d in enumerate(FRESH_TAPS):
                        ky, kx = 2 - d[0], 2 - d[1]
                        sap = _ap(cwn, 0, ky * 5 + kx,
                                  [[192 * 25, 128], [25, 192]]).bitcast(F32)
                        nc.vector.tensor_copy(WdNf[f][0:128, mi, 0:192], sap)

                # Pool queue: ep_w1..5 (rotation stalls stay off HWDGE queues)
                WN = {li: pro.tile([128, cdiv(DIMS[li + 1], 128), DIMS[li]], BF16,
                                   tag=f"WN{li}", name=f"WN{li}", bufs=1)
                      for li in range(1, 5)}
                def load_and_transpose(li):
                    n_out, n_in = DIMS[li + 1], DIMS[li]
                    wnat = pro.tile([128, 4, 640], F32, tag="wnatS", name="wnatS", bufs=2)
                    for mi, (ms, mw) in enumerate(chunks_of(n_out)):
                        nc.gpsimd.dma_start(wnat[0:mw, mi, 0:n_in],
                                            di[f'ep_w{li}'].ap()[ms:ms + mw])
                    for ci, (cs, cww) in enumerate(chunks_of(n_in)):
                        mchunks = list(enumerate(chunks_of(n_out)))
                        for mb in range(cdiv(len(mchunks), 4)):
                            mcb = mchunks[mb * 4:(mb + 1) * 4]
                            pt = prps.tile([128, 4, 128], F32, tag="tp")
                            for k, (mi, (ms, mw)) in enumerate(mcb):
                                nc.tensor.transpose(pt[0:cww, k, 0:mw],
                                                    wnat[0:mw, mi, cs:cs + cww],
                                                    ident[0:mw, 0:mw])
                            ms0 = mcb[0][1][0]
                            tw = sum(mw for _, (ms, mw) in mcb)
                            span = pt[:, 0:len(mcb), :].rearrange("p a b -> p (a b)")[0:cww, 0:tw]
                            evict(WT[li][0:cww, ci, ms0:ms0 + tw], span, scale=0.99)
                    if li <= 4:
                        for mi, (ms, mw) in enumerate(chunks_of(n_out)):
                            evict(WN[li][0:mw, mi, 0:n_in], wnat[0:mw, mi, 0:n_in])
                for li in range(1, 6):
                    load_and_transpose(li)

                # GT[f] = (W0c @ Wd_tap)^T = WdN^T-compose (all bf16)
                for f in range(2):
                    for mc, (cs, cww) in enumerate(chunks_of(192)):
                        for nh in range(2):
                            gp = prps.tile([128, 512], F32, tag="dps")
                            for ki in range(3):
                                nc.tensor.matmul(gp[0:cww, 0:320],
                                                 WdNf[f][0:128, ki, cs:cs + cww],
                                                 W0cT[0:128, ki, nh * 320:(nh + 1) * 320],
                                                 start=(ki == 0), stop=(ki == 2),
                                                 skip_group_check=True)
                            evict(GT[f][0:cww, mc, nh * 320:(nh + 1) * 320],
                                  gp[0:cww, 0:320])

                # ---- relu-decomposition composites ----
                # PT[l] = 0.0099*(W_{l+1} W_l)^T  (WT tiles carry 0.99 each)
                for li in range(1, 5):
                    nN = DIMS[li + 2]
                    for m, (ms, mw) in enumerate(chunks_of(DIMS[li])):
                        gp = prps.tile([128, 512], F32, tag="dps")
                        kch = chunks_of(DIMS[li + 1])
                        for k, (ks, kw) in enumerate(kch):
                            nc.tensor.matmul(gp[0:mw, 0:nN],
                                             WN[li][0:kw, k, ms:ms + mw],
                                             WT[li + 1][0:kw, k, 0:nN],
                                             start=(k == 0), stop=(k == len(kch) - 1),
                                             skip_group_check=True)
                        evict(PT[li][0:mw, m, 0:nN], gp[0:mw, 0:nN],
                              scale=0.0099 / (0.99 * 0.99))

                # F1T/C1T = 0.01*(W1 W0f/c)^T  (WT[1] carries 0.99)
                for nat, dstT in ((W0fN, F1T), (W0cN, C1T)):
                    for m, (ms, mw) in enumerate(chunks_of(384)):
                        gp = prps.tile([128, 512], F32, tag="dps")
                        for k in range(5):
                            nc.tensor.matmul(gp[0:mw, 0:512],
                                             nat[0:128, k, ms:ms + mw],
                                             WT[1][0:128, k, 0:512],
                                             start=(k == 0), stop=(k == 4),
                                             skip_group_check=True)
                        evict(dstT[0:mw, m, 0:512], gp[0:mw, 0:512],
                              scale=0.01 / 0.99)

                # G1T[f] = (C1 @ Wd_tap)^T = WdN-compose with C1T (C1T has the 0.01)
                for f in range(2):
                    for mc, (cs, cww) in enumerate(chunks_of(192)):
                        gp = prps.tile([128, 512], F32, tag="dps")
                        for ki in range(3):
                            nc.tensor.matmul(gp[0:cww, 0:512],
                                             WdNf[f][0:128, ki, cs:cs + cww],
                                             C1T[0:128, ki, 0:512],
                                             start=(ki == 0), stop=(ki == 2),
                                             skip_group_check=True)
                        evict(G1T[f][0:cww, mc, 0:512], gp[0:cww, 0:512])

                # wbrow[l] = (0.01/0.99)*W_l b_{l-1} as a row (K=1 MM transposes)
                bcol = pro.tile([128, 6, 1], BF16, tag="bcol", bufs=1)
                wbc = pro.tile([128, 4, 1], BF16, tag="wbc", bufs=2)
                onne = pro.tile([1, 1], BF16, tag="onne", bufs=1)
                nc.vector.memset(onne[:], 1.0)
                for li in range(1, 6):
                    kch = chunks_of(DIMS[li])
                    bp = prps.tile([128, 6, 1], F32, tag="dps", name="bp")
                    for k, (ks, kw) in enumerate(kch):
                        nc.tensor.matmul(bp[0:kw, k, 0:1],
                                         brow[li - 1][0:1, ks:ks + kw],
                                         onne[0:1, 0:1],
                                         start=True, stop=True, skip_group_check=True)
                    nc.vector.tensor_copy(bcol[:, 0:len(kch), 0:1], bp[:, 0:len(kch), 0:1])
                    wbp = prps.tile([128, 4, 1], F32, tag="dps", name="wbp")
                    mch = chunks_of(DIMS[li + 1])
                    for m, (ms, mw) in enumerate(mch):
                        for k, (ks, kw) in enumerate(kch):
                            nc.tensor.matmul(wbp[0:mw, m, 0:1],
                                             WT[li][0:kw, k, ms:ms + mw],
                                             bcol[0:kw, k, 0:1],
                                             start=(k == 0), stop=(k == len(kch) - 1),
                                             skip_group_check=True)
                    nc.vector.tensor_copy(wbc[:, 0:len(mch), 0:1], wbp[:, 0:len(mch), 0:1])
                    rp = prps.tile([128, 512], F32, tag="dps", name="rp")
                    for m, (ms, mw) in enumerate(mch):
                        nc.tensor.matmul(rp[0:1, 0:mw], wbc[0:mw, m, 0:1],
                                         identB[0:mw, 0:mw],
                                         start=True, stop=True, skip_group_check=True)
                        evict(wbrow[li][0:1, ms:ms + mw], rp[0:1, 0:mw],
                              scale=0.01 / 0.99)

            # ============ PROLOGUE 3: conv2 weights ============
            with tc.tile_pool(name="proC", bufs=1) as pro, \
                 tc.tile_pool(name="prpsC", bufs=2, space="PSUM") as prps:
                cw2st = []
                for mi in range(3):
                    cw2s = pro.tile([128, 288 * 9], F32, tag=f"cw2s{mi}", name=f"cw2s{mi}")
                    nc.sync.dma_start(
                        cw2s[:],
                        di['hs_cw2'].ap()[mi * 128:(mi + 1) * 128]
                        .rearrange("o c kh kw -> o (c kh kw)"))
                    cw2st.append(cw2s)
                tp_count = [0]
                def evict(dst_ap, src_ap):
                    if tp_count[0] % 2 == 0:
                        nc.vector.tensor_copy(dst_ap, src_ap)
                    else:
                        nc.scalar.activation(dst_ap, src_ap,
                                             mybir.ActivationFunctionType.Copy)
                    tp_count[0] += 1
                for mi in range(3):
                    nc.vector.memset(cw2T[32:64, mi, 2, :], 0.0)
                    nc.gpsimd.memset(cw2T[64:128, mi, 2, :], 0.0)
                    for si, (ss, sw) in enumerate(chunks_of(288)):
                        for kb in range(3):
                            ks = list(range(9))[kb * 4:(kb + 1) * 4]
                            if not ks:
                                continue
                            pt = prps.tile([128, 4, 128], F32, tag="tp")
                            for kk, k in enumerate(ks):
                                sap = _ap(cw2st[mi], 0, ss * 9 + k, [[288 * 9, 128], [9, sw]])
                                nc.tensor.transpose(pt[0:sw, kk, 0:128], sap, ident[:])
                            span = pt[:, 0:len(ks), :].rearrange("p a b -> p (a b)")[0:sw, 0:len(ks) * 128]
                            evict(_ap(cw2T, (mi * 3 + si) * 9 * 128, ks[0] * 128,
                                      [[3 * 3 * 9 * 128, sw], [1, len(ks) * 128]]),
                                  span)

            # conv2 rows 0..1 upfront; rest streamed into the wavefront steps
            for mi in range(3):
                emit_conv2_unit(nc, pps, cw2T, m2, fm1, b_c2, mi, 0, 1, 0, 27)

            # ================= SEQUENTIAL PHASE =================
            # conv2 streaming: 2-row units (mi, rb), rows [2rb, 2rb+2)
            pf_units = [(mi, rb) for rb in range(1, 16) for mi in range(3)]
            pf_state = {"u": 0, "k": 0, "ps": None}

            def pfill(nmm):
                while nmm > 0 and pf_state["u"] < len(pf_units):
                    mi, rb = pf_units[pf_state["u"]]
                    take = min(nmm, 27 - pf_state["k"])
                    ps = emit_conv2_unit(nc, pps, cw2T, m2, fm1, b_c2, mi,
                                         2 * rb, 2 * rb + 2,
                                         pf_state["k"], pf_state["k"] + take,
                                         ps=pf_state["ps"])
                    pf_state["ps"] = ps
                    pf_state["k"] += take
                    nmm -= take
                    if pf_state["k"] == 27:
                        pf_state["u"] += 1
                        pf_state["k"] = 0
                        pf_state["ps"] = None

            def ydiag_ap(img, i0, j0, kw, c, B):
                """[kw, B] wavefront-diagonal AP into padded img tile chunk c."""
                off = (i0 + 2) * WP + (j0 + 2)
                return _ap(img, c * HP * WP, off, [[2 * HP * WP, kw], [DIAG, B]])

            def emit_seed2(pt, slot, brow_ap, mw, B):
                nc.tensor.matmul(pt[0:mw, slot, 0:B], brow_ap, ones[0:1, 0:B],
                                 start=True, stop=False, skip_group_check=True)

            def emit_old_ctx(t1):
                """ctx_b seed for step t1's ctx -> zc[t1%2][5:8]; return tap MM list."""
                s1 = t1 % 2
                i_lo, B, j_lo = step_geom(t1)
                ms_list = chunks_of(384)
                for m, (ms, mw) in enumerate(ms_list):
                    emit_seed2(zc[s1], 5 + m, ctxb[0:1, ms:ms + mw], mw, B)
                mms = []
                for ti, (dy, dx) in enumerate(OLD_TAPS):
                    for c, (cs, kw) in enumerate(chunks_of(192)):
                        for m, (ms, mw) in enumerate(ms_list):
                            mms.append((ti, dy, dx, c, cs, kw, m, ms, mw))
                return i_lo, B, j_lo, mms

            # prologue part of step 0's z0/ctx accumulation
            def emit_z0_pre(t1):
                """seeds + f-part + (later) ctx-part for z0 of step t1 -> zc[t1%2][0:5]"""
                s1 = t1 % 2
                i_lo, B, j_lo = step_geom(t1)
                for m, (ms, mw) in enumerate(chunks_of(640)):
                    emit_seed2(zc[s1], m, brow[0][0:1, ms:ms + mw], mw, B)
                for k in range(3):
                    for m, (ms, mw) in enumerate(chunks_of(640)):
                        rhs = _ap(fm1, k * NPIX, i_lo * W + j_lo,
                                  [[3 * NPIX, 128], [W - 3, B]])
                        nc.tensor.matmul(zc[s1][0:mw, m, 0:B],
                                         W0fT[0:128, k, ms:ms + mw], rhs,
                                         start=False, stop=False, skip_group_check=True)

            def emit_z0_ctx(t1):
                s1 = t1 % 2
                i_lo, B, j_lo = step_geom(t1)
                for k in range(3):
                    for m, (ms, mw) in enumerate(chunks_of(640)):
                        nc.tensor.matmul(zc[s1][0:mw, m, 0:B],
                                         W0cT[0:128, k, ms:ms + mw],
                                         Xc[0:128, k, 0:B],
                                         start=False, stop=False, skip_group_check=True)

            def emit_old_mms(t1, geom, mms):
                i_lo, B, j_lo = geom
                s1 = t1 % 2
                for (ti, dy, dx, c, cs, kw, m, ms, mw) in mms:
                    rhs = ydiag_ap(Yimg, i_lo - dy, j_lo - dx, kw, c, B)
                    last = (ti == len(OLD_TAPS) - 1) and (c == 1)
                    nc.tensor.matmul(zc[s1][0:mw, 5 + m, 0:B],
                                     WdT[OLD_TAPS[ti]][0:kw, c, ms:ms + mw], rhs,
                                     start=False, stop=last, skip_group_check=True)

            # --- step 0 pre-work (its sources are all zero borders) ---
            g0 = emit_old_ctx(0)
            emit_old_mms(0, (g0[0], g0[1], g0[2]), g0[3])
            i_lo0, B0, j_lo0 = step_geom(0)
            nc.vector.tensor_copy(Xc[:, 0:3, 0:B0], zc[0][:, 5:8, 0:B0])
            emit_z0_pre(0)
            emit_z0_ctx(0)

            KCHW = {li: chunks_of(DIMS[li]) for li in range(1, 6)}
            MCHW = {li: chunks_of(DIMS[li + 1]) for li in range(0, 6)}

            for t in range(nsteps):
                s = t % 2
                s1 = (t + 1) % 2
                i_lo, B, j_lo = step_geom(t)
                have_next = t + 1 < nsteps
                if have_next:
                    i_lo1, B1, j_lo1 = step_geom(t + 1)

                # ---- fresh taps -> z0 (critical) ----
                for m, (ms, mw) in enumerate(MCHW[0]):
                    for f in range(2):
                        dy, dx = FRESH_TAPS[f]
                        for c, (cs, kw) in enumerate(chunks_of(192)):
                            rhs = ydiag_ap(Yimg, i_lo - dy, j_lo - dx, kw, c, B)
                            nc.tensor.matmul(zc[s][0:mw, m, 0:B],
                                             GT[f][0:kw, c, ms:ms + mw], rhs,
                                             start=False,
                                             stop=(f == 1 and c == 1),
                                             skip_group_check=True)

                # z1 "early" contributions + all z seeds (run inside r0's hop)
                for li in range(1, 6):
                    for m, (ms, mw) in enumerate(MCHW[li]):
                        emit_seed2(zs, ZOFF[li] + m, brow[li][0:1, ms:ms + mw], mw, B)
                        nc.tensor.matmul(zs[0:mw, ZOFF[li] + m, 0:B],
                                         wbrow[li][0:1, ms:ms + mw], ones[0:1, 0:B],
                                         start=False, stop=False, skip_group_check=True)
                for m, (ms, mw) in enumerate(MCHW[1]):
                    for f in range(2):   # G1 fresh (0.01 level)
                        dy, dx = FRESH_TAPS[f]
                        for c, (cs, kw) in enumerate(chunks_of(192)):
                            rhs = ydiag_ap(Yimg, i_lo - dy, j_lo - dx, kw, c, B)
                            nc.tensor.matmul(zs[0:mw, ZOFF[1] + m, 0:B],
                                             G1T[f][0:kw, c, ms:ms + mw], rhs,
                                             start=False, stop=False,
                                             skip_group_check=True)
                    for k in range(3):   # C1 * ctx_old, F1 * f
                        nc.tensor.matmul(zs[0:mw, ZOFF[1] + m, 0:B],
                                         C1T[0:128, k, ms:ms + mw], Xc[0:128, k, 0:B],
                                         start=False, stop=False, skip_group_check=True)
                        rhs = _ap(fm1, k * NPIX, i_lo * W + j_lo,
                                  [[3 * NPIX, 128], [W - 3, B]])
                        nc.tensor.matmul(zs[0:mw, ZOFF[1] + m, 0:B],
                                         F1T[0:128, k, ms:ms + mw], rhs,
                                         start=False, stop=False, skip_group_check=True)

                # ---- r0 = relu(z0) (critical DVE) ----
                nc.vector.tensor_scalar_max(xs[1][:, 0:5, 0:B], zc[s][:, 0:5, 0:B], 0.0)

                # old-ctx for t+1 (fillers)
                old_mms = []
                if have_next:
                    g = emit_old_ctx(t + 1)
                    old_mms = g[3]
                    geom1 = (g[0], g[1], g[2])

                # ---- MLP layers 1..5 ----
                for li in range(1, 6):
                    # fillers in the gap before this layer's critical MMs
                    if li == 2 and have_next:
                        emit_old_mms(t + 1, geom1, old_mms[:33])
                    elif li == 3 and have_next:
                        emit_old_mms(t + 1, geom1, old_mms[33:])
                    elif li == 4 and have_next:
                        nc.scalar.activation(Xc[:, 0:3, 0:B1], zc[s1][:, 5:8, 0:B1],
                                             mybir.ActivationFunctionType.Copy)
                        emit_z0_pre(t + 1)
                        pfill(10)
                    elif li == 5 and have_next:
                        pfill(14)
                    # pair term P_{li-1} * r_{li-2} -> z_li (off critical path)
                    if li >= 2:
                        pl = li - 1
                        kch = chunks_of(DIMS[pl])
                        for m, (ms, mw) in enumerate(MCHW[li]):
                            for k, (ks, kw) in enumerate(kch):
                                nc.tensor.matmul(zs[0:mw, ZOFF[li] + m, 0:B],
                                                 PT[pl][0:kw, k, ms:ms + mw],
                                                 xs[pl][0:kw, k, 0:B],
                                                 start=False, stop=False,
                                                 skip_group_check=True)
                    # critical: 0.99*W_li * r_{li-1}
                    kch = KCHW[li]
                    for m, (ms, mw) in enumerate(MCHW[li]):
                        for k, (ks, kw) in enumerate(kch):
                            nc.tensor.matmul(zs[0:mw, ZOFF[li] + m, 0:B],
                                             WT[li][0:kw, k, ms:ms + mw],
                                             xs[li][0:kw, k, 0:B],
                                             start=False, stop=(k == len(kch) - 1),
                                             skip_group_check=True)
                    if li < 5:
                        nch = len(MCHW[li])
                        nc.vector.tensor_scalar_max(
                            xs[li + 1][:, 0:nch, 0:B],
                            zs[:, ZOFF[li]:ZOFF[li] + nch, 0:B], 0.0)

                # ---- Y = z5 + w_hat (critical DVE) ----
                off = (i_lo + 2) * WP + (j_lo + 2)
                ydst = _ap(Yimg, 0, off, [[2 * HP * WP, 128], [HP * WP, 2], [DIAG, B]])
                ywim = _ap(wimg, 0, i_lo * W + j_lo,
                           [[2 * NPIX, 128], [NPIX, 2], [W - 3, B]])
                nc.vector.tensor_tensor(ydst, zs[:, 12:14, 0:B], ywim,
                                        mybir.AluOpType.add)
                # late filler: ctx->z0 for t+1
                if have_next:
                    emit_z0_ctx(t + 1)

            # ================= EPILOGUE =================
            with tc.tile_pool(name="epi", bufs=1) as epi:
                Yimg32 = epi.tile([128, 2, NPIX], F32)
                src = _ap(Yimg, 0, 2 * WP + 2,
                          [[2 * HP * WP, 128], [HP * WP, 2], [WP, H], [1, W]])
                dst = _ap(Yimg32, 0, 0,
                          [[2 * NPIX, 128], [NPIX, 2], [W, H], [1, W]])
                nc.vector.tensor_copy(dst, src)
                ov = out.ap()[0]
                for ci, (cs, cw) in enumerate(chunks_of(192)):
                    nc.sync.dma_start(
                        ov[cs:cs + cw],
                        Yimg32[0:cw, ci, :].rearrange("p (h w) -> p h w", h=H))

    nc.compile()
    return nc


def emit_conv2_unit(nc, pps, cw2T, m2, fm1, b_c2, mi, r0, r1, k0, k1, ps=None):
    """Emit conv2 MMs [k0, k1) for out-chunk mi over rows [r0, r1);
    27 MMs per unit. MM index kk = k * 3 + si."""
    F32 = mybir.dt.float32
    nr = r1 - r0
    if ps is None:
        ps = pps.tile([128, 384], F32, tag="cps", name="cps")
    for kk in range(k0, k1):
        k, si = kk // 3, kk % 3
        ky, kx = k // 3, k % 3
        lhsT = _ap(cw2T, (mi * 3 + si) * 9 * 128, k * 128,
                   [[3 * 3 * 9 * 128, 128], [1, 128]])
        rhs = _ap(m2, si * 34 * 50, (ky + r0) * 50 + kx,
                  [[3 * 34 * 50, 128], [50, nr], [1, 48]])
        nc.tensor.matmul(ps[:, 0:nr * 48], lhsT, rhs,
                         start=(kk == 0), stop=(kk == 26), skip_group_check=True)
    if k1 == 27:
        nc.scalar.activation(fm1[:, mi, r0 * 48:r1 * 48], ps[:, 0:nr * 48],
                             mybir.ActivationFunctionType.Identity,
                             bias=b_c2[:, mi][:, None], alpha=0.0)
    return ps


_NC_CACHE = {}


def kernel(**inputs):
    from concourse.bass_utils import run_bass_kernel_spmd
    key = "full"
    if key not in _NC_CACHE:
        _NC_CACHE[key] = build()
    nc = _NC_CACHE[key]
    in_map = {k: np.ascontiguousarray(np.asarray(v, dtype=np.float32))
              for k, v in inputs.items()}
    res = run_bass_kernel_spmd(nc, [in_map] * 8, core_ids=list(range(8)))
    return res.results[0]['out']


if __name__ == "__main__":
    t = build(nsteps=int(sys.argv[1]) if len(sys.argv) > 1 else NSTEPS)
    print("build ok")
    from concourse.timeline_sim import TimelineSim
    est = TimelineSim(t).simulate()
    print(f"HW exec time: {est:.0f} ns")


# revision 39
# speedup vs baseline: 1.0124x; 1.0124x over previous
"""Trainium2 Bass kernel for nn_BEE_Bin2Symbol (hyper-decoder + masked-conv
autoregressive MLP decoder).

Architecture (v2, latency-oriented):
- Sequential phase runs all GEMMs in [C_out-on-partitions, B-pixels-free]
  orientation (weights stationary as lhsT, activations moving): matmul cost
  scales with B<=16, transposes disappear, and each layer's nonlinearity is a
  single DVE scalar_tensor_tensor op  max(0.01*z, z)  reading PSUM directly.
- 140 slope-3 anti-diagonal wavefronts. Per step the critical chain is
  fresh-tap MMs -> lrelu(z0) -> W1 MMs -> lrelu(z1) -> ... -> z5 -> Y-add.
  Everything else (old-tap ctx GEMMs, f-part, bias seeds, phase-P conv
  streaming) is issued into the PE gaps between the chain's engine hops.
- Fresh taps (age-1) use precomposed G = W0c @ Wd_tap so they feed z0
  directly; old taps accumulate a ctx PSUM a step ahead, evicted by the
  scalar engine (ACT) off the critical path.
- All sequential-phase weights/activations in bf16 (PSUM accumulate f32).
- Hyper-decoder (2 stride-2 deconvs + 3x3 conv) in f32r as phase-decomposed
  GEMMs; conv2's last 3 row-blocks stream into the first ~15 wavefront steps.
"""
import sys, os
sys.path.insert(0, "/opt/trn_rl_repo")

import numpy as np

import concourse.bass as bass
import concourse.bacc as bacc
import concourse.mybir as mybir
import concourse.tile as tile
from concourse.masks import make_identity

F32 = mybir.dt.float32
F32R = mybir.dt.float32r
BF16 = mybir.dt.bfloat16

H, W = 32, 48
HP, WP = H + 4, W + 4            # padded image 36 x 52
NPIX = H * W
NSTEPS = 3 * (H - 1) + (W - 1) + 1   # 140
DIAG = WP - 3                    # 49: wavefront-diagonal stride in padded img

# taps (dy, dx): source pixel = (i-dy, j-dx); ctx_w index (ky,kx) = (2-dy, 2-dx)
TAPS = [(2, 2), (2, 1), (2, 0), (2, -1), (2, -2),
        (1, 2), (1, 1), (1, 0), (1, -1), (1, -2),
        (0, 1), (0, 2)]
FRESH_TAPS = [(1, -2), (0, 1)]                # age-1 taps (need step t-1)
OLD_TAPS = [d for d in TAPS if d not in FRESH_TAPS]

DIMS = [768, 640, 512, 384, 320, 256, 192]   # MLP dims; layer l: DIMS[l]->DIMS[l+1]
# z1..z5 chunk offsets inside the shared zs psum tile [128, 14, 16]
ZOFF = {1: 0, 2: 4, 3: 7, 4: 10, 5: 12}


def cdiv(a, b):
    return (a + b - 1) // b


def chunks_of(n, c=128):
    return [(s, min(c, n - s)) for s in range(0, n, c)]


def _ap(tile_ap, slot_off, elem_off, plist):
    """Build a custom AP into a [128, S, F]-shaped sbuf/psum tile."""
    base = tile_ap[:]
    return bass.AP(base.tensor, base.offset + slot_off + elem_off, plist)


def step_geom(t):
    i_lo = max(0, cdiv(t - (W - 1), 3))
    i_hi = min(H - 1, t // 3)
    return i_lo, i_hi - i_lo + 1, t - 3 * i_lo


def build(nsteps=NSTEPS):
    nc = bacc.Bacc()

    # ---------------- DRAM I/O ----------------
    di = {}
    di['z_hat'] = nc.dram_tensor('z_hat', [1, 192, 8, 12], F32, kind="ExternalInput")
    di['w_hat'] = nc.dram_tensor('w_hat', [1, 192, 32, 48], F32, kind="ExternalInput")
    di['hs_dw0'] = nc.dram_tensor('hs_dw0', [192, 192, 5, 5], F32, kind="ExternalInput")
    di['hs_db0'] = nc.dram_tensor('hs_db0', [192], F32, kind="ExternalInput")
    di['hs_dw1'] = nc.dram_tensor('hs_dw1', [192, 288, 5, 5], F32, kind="ExternalInput")
    di['hs_db1'] = nc.dram_tensor('hs_db1', [288], F32, kind="ExternalInput")
    di['hs_cw2'] = nc.dram_tensor('hs_cw2', [384, 288, 3, 3], F32, kind="ExternalInput")
    di['hs_cb2'] = nc.dram_tensor('hs_cb2', [384], F32, kind="ExternalInput")
    di['ctx_w'] = nc.dram_tensor('ctx_w', [384, 192, 5, 5], F32, kind="ExternalInput")
    di['ctx_b'] = nc.dram_tensor('ctx_b', [384], F32, kind="ExternalInput")
    for li in range(6):
        di[f'ep_w{li}'] = nc.dram_tensor(f'ep_w{li}', [DIMS[li + 1], DIMS[li]], F32,
                                         kind="ExternalInput")
        di[f'ep_b{li}'] = nc.dram_tensor(f'ep_b{li}', [DIMS[li + 1]], F32,
                                         kind="ExternalInput")
    out = nc.dram_tensor('out', [1, 192, 32, 48], F32, kind="ExternalOutput")

    with tile.TileContext(nc) as tc:
        with tc.tile_pool(name="pp", bufs=1) as pp, \
             tc.tile_pool(name="pps", bufs=1, space="PSUM") as pps:

            ident = pp.tile([128, 128], F32)
            make_identity(nc, ident[:])

            # ---------- persistent state ----------
            Yimg = pp.tile([128, 2, HP * WP], BF16)   # decoded image (padded)
            nc.vector.memset(Yimg[:], 0.0)
            wimg = pp.tile([128, 2, NPIX], F32)       # w_hat residual (compact)
            fm1 = pp.tile([128, 3, NPIX], BF16)       # conv2 output [384, 1536]
            m2 = pp.tile([128, 3, 34 * 50], BF16)     # deconv1 out (padded 34x50)
            nc.gpsimd.memset(m2[:], 0.0)
            cw2T = pp.tile([128, 3, 3, 9 * 128], BF16)  # [cin, mi, si, k*128+o]

            # transposed weights (bf16)
            W0fT = pp.tile([128, 3, 640], BF16)
            W0cT = pp.tile([128, 3, 640], BF16)
            WT = {}
            for li in range(1, 6):
                WT[li] = pp.tile([128, cdiv(DIMS[li], 128), DIMS[li + 1]], BF16,
                                 tag=f"W{li}T", name=f"W{li}T")
            GT = [pp.tile([128, 2, 640], BF16, tag=f"GT{k}", name=f"GT{k}") for k in range(2)]
            # relu-decomposition composites (0.01*z linear flow)
            PT = {}   # PT[l] = 0.0099*(W_{l+1} W_l)^T  -> feeds z_{l+1} from r_{l-1}
            for li in range(1, 5):
                PT[li] = pp.tile([128, cdiv(DIMS[li], 128), DIMS[li + 2]], BF16,
                                 tag=f"PT{li}", name=f"PT{li}")
            F1T = pp.tile([128, 3, 512], BF16)   # 0.01*(W1 W0f)^T
            C1T = pp.tile([128, 3, 512], BF16)   # 0.01*(W1 W0c)^T
            G1T = [pp.tile([128, 2, 512], BF16, tag=f"G1T{k}", name=f"G1T{k}")
                   for k in range(2)]            # 0.01*(W1 G_tap)^T
            wbrow = [None] + [pp.tile([1, DIMS[li + 1]], BF16, tag=f"wb{li}", name=f"wb{li}")
                              for li in range(1, 6)]   # 0.01*W_l b_{l-1} rows
            identB = pp.tile([128, 128], BF16)
            make_identity(nc, identB[:])
            WdT = {}
            for d in OLD_TAPS:
                WdT[d] = pp.tile([128, 2, 384], BF16, tag=f"Wd{d[0]}_{d[1]}", name=f"Wd{d[0]}_{d[1]}")

            # bias rows (lhsT for K=1 seed matmuls)
            brow = [pp.tile([1, DIMS[li + 1]], BF16, tag=f"b{li}", name=f"b{li}") for li in range(6)]
            ctxb = pp.tile([1, 384], BF16)
            ones = pp.tile([1, 16], BF16)
            nc.vector.memset(ones[:], 1.0)

            # sequential-phase activations (single tiles; WAR handled by sems)
            xs = {li: pp.tile([128, cdiv(DIMS[li], 128), 16], BF16, tag=f"x{li}", name=f"x{li}")
                  for li in range(1, 6)}
            Xc = pp.tile([128, 3, 16], BF16)          # evicted old-ctx

            # persistent psum: z0+ctx ring (2 banks), mlp zs (1), conv stream (1)
            zc = [pps.tile([128, 8, 16], F32, tag=f"zc{s}", name=f"zc{s}") for s in range(2)]
            zs = pps.tile([128, 14, 16], F32, tag="zs")
            nc.vector.memset(zs[:], 0.0)
            nc.vector.memset(zc[0][:], 0.0)
            nc.vector.memset(zc[1][:], 0.0)

            # ============ PROLOGUE 1: hyper-decoder (DMA priority) ============
            with tc.tile_pool(name="proB", bufs=2) as pro, \
                 tc.tile_pool(name="prpsB", bufs=2, space="PSUM") as prps:

                # SP queue: deconv0 weights lead everything
                dw0t = []
                for mi, (ms, mw) in enumerate(chunks_of(192)):
                    dw = pro.tile([128, 2, 128 * 25], F32R, tag="dw", name="dw", bufs=2)
                    for ci, (cs, cww) in enumerate(chunks_of(192)):
                        nc.sync.dma_start(
                            dw[0:cww, ci, 0:mw * 25],
                            di['hs_dw0'].ap()[cs:cs + cww, ms:ms + mw]
                            .rearrange("c o kh kw -> c (o kh kw)").bitcast(F32R))
                    dw0t.append(dw)

                # Pool/SWDGE queue: small loads (zp first - deconv0 input)
                zp = pro.tile([128, 2, 10 * 14], F32R, tag="zp", bufs=1)
                nc.vector.memset(zp[:].bitcast(F32), 0.0)
                zv = di['z_hat'].ap()[0]
                for ci, (s, cw) in enumerate(chunks_of(192)):
                    dst = _ap(zp, ci * 140, 14 + 1, [[2 * 140, cw], [14, 8], [1, 12]])
                    nc.gpsimd.dma_start(dst, zv[s:s + cw].bitcast(F32R))

                def load_bias_col(name, n):
                    nch = cdiv(n, 128)
                    t = pp.tile([128, nch], F32, tag=f"b_{name}", name=f"b_{name}")
                    nc.vector.memset(t[:], 0.0)
                    for ci, (s, w_) in enumerate(chunks_of(n)):
                        nc.gpsimd.dma_start(t[0:w_, ci:ci + 1], di[name][s:s + w_][:, None])
                    return t
                b_d0 = load_bias_col('hs_db0', 192)
                b_d1 = load_bias_col('hs_db1', 288)
                b_c2 = load_bias_col('hs_cb2', 384)

                def load_brow(dst, dram, n):
                    st = pro.tile([1, 640], F32, tag="brs", name="brs", bufs=1)
                    nc.gpsimd.dma_start(st[0:1, 0:n], dram.ap()[None, :])
                    nc.vector.tensor_copy(dst[0:1, 0:n], st[0:1, 0:n])
                for li in range(6):
                    load_brow(brow[li], di[f'ep_b{li}'], DIMS[li + 1])
                load_brow(ctxb, di['ctx_b'], 384)

                whv = di['w_hat'].ap()[0]
                for ci, (s, cw) in enumerate(chunks_of(192)):
                    nc.gpsimd.dma_start(
                        wimg[0:cw, ci, :].rearrange("p (h w) -> p h w", h=H),
                        whv[s:s + cw])

                m1 = pro.tile([128, 2, 18 * 26], F32R, tag="m1", bufs=1)
                nc.vector.memset(m1[:].bitcast(F32), 0.0)

                def deconv_chunk(inp_t, inp_hw, w_t, cin, mw, mi, out_t, bias_t):
                    hi, wi = inp_hw
                    ip_w = wi + 2
                    op_w = 2 * wi + 2
                    for py in range(2):
                        for px in range(2):
                            ps = prps.tile([128, 16 * 24], F32, tag="dps")
                            first = True
                            taps = [(u, v) for u in range(py, 5, 2) for v in range(px, 5, 2)]
                            for ti, (u, v) in enumerate(taps):
                                dy = (py + 2 - u) // 2
                                dx = (px + 2 - v) // 2
                                for ci, (cs, cww) in enumerate(chunks_of(cin)):
                                    lhsT = _ap(w_t, ci * 128 * 25, u * 5 + v,
                                               [[2 * 128 * 25, cww], [25, mw]])
                                    rhs = _ap(inp_t, ci * (hi + 2) * ip_w,
                                              (1 + dy) * ip_w + (1 + dx),
                                              [[2 * (hi + 2) * ip_w, cww], [ip_w, hi], [1, wi]])
                                    last = (ti == len(taps) - 1) and (ci == len(chunks_of(cin)) - 1)
                                    nc.tensor.matmul(ps[0:mw, 0:hi * wi], lhsT, rhs,
                                                     start=first, stop=last,
                                                     skip_group_check=True)
                                    first = False
                            dst = _ap(out_t, mi * (2 * hi + 2) * op_w,
                                      (py + 1) * op_w + (px + 1),
                                      [[out_t.shape[1] * (2 * hi + 2) * op_w, mw],
                                       [2 * op_w, hi], [2, wi]])
                            nc.scalar.activation(
                                dst, ps[0:mw, 0:hi * wi].rearrange("p (a b) -> p a b", a=hi),
                                mybir.ActivationFunctionType.Lrelu,
                                bias=bias_t[0:mw, mi][:, None], alpha=0.01)

                # ACT queue: dw1 (so SP-queue dw rotation can't block it)
                dw1t = []
                for mi, (ms, mw) in enumerate(chunks_of(288)):
                    dw = pro.tile([128, 2, 128 * 25], F32R, tag="dw", name="dw1", bufs=2)
                    for ci, (cs, cww) in enumerate(chunks_of(192)):
                        nc.scalar.dma_start(
                            dw[0:cww, ci, 0:mw * 25],
                            di['hs_dw1'].ap()[cs:cs + cww, ms:ms + mw]
                            .rearrange("c o kh kw -> c (o kh kw)").bitcast(F32R))
                    dw1t.append(dw)

                # deconv0: z[192,8,12] -> m1[192,16,24]
                for mi, (ms, mw) in enumerate(chunks_of(192)):
                    deconv_chunk(zp, (8, 12), dw0t[mi], 192, mw, mi, m1, b_d0)

                # deconv1: m1[192,16,24] -> m2[288,32,48]
                for mi, (ms, mw) in enumerate(chunks_of(288)):
                    deconv_chunk(m1, (16, 24), dw1t[mi], 192, mw, mi, m2, b_d1)

            # ============ PROLOGUE 2: weight transposes ============
            with tc.tile_pool(name="pro", bufs=2) as pro, \
                 tc.tile_pool(name="prps", bufs=2, space="PSUM") as prps:

                tp_count = [0]
                def evict(dst_ap, src_ap, scale=None):
                    if scale is None and tp_count[0] % 2 == 0:
                        nc.vector.tensor_copy(dst_ap, src_ap)
                    elif scale is None:
                        nc.scalar.activation(dst_ap, src_ap,
                                             mybir.ActivationFunctionType.Copy)
                    elif tp_count[0] % 2 == 0:
                        nc.vector.tensor_scalar_mul(dst_ap, src_ap, float(scale))
                    else:
                        nc.scalar.activation(dst_ap, src_ap,
                                             mybir.ActivationFunctionType.Copy,
                                             scale=float(scale))
                    tp_count[0] += 1

                def wnat0_half(h, eng=None):
                    t = pro.tile([128, 5, 768], F32, tag="wnatS", name="wnat0",
                                 bufs=1)
                    e = eng or nc.sync
                    for mi, (ms, mw) in enumerate(chunks_of(640)):
                        if mi // 3 != h:
                            continue
                        e.dma_start(t[0:mw, mi % 3, 0:768],
                                    di['ep_w0'].ap()[ms:ms + mw])
                    return t


                # SP queue (free after dw0): cwn0, wnat0-h1, cw2s 0/1
                def cwn_load(mi, eng):
                    t = pro.tile([128, 192 * 25], F32R, tag="cwn", name="cwn", bufs=2)
                    eng.dma_start(
                        t[:],
                        di['ctx_w'].ap()[mi * 128:(mi + 1) * 128]
                        .rearrange("o c kh kw -> o (c kh kw)").bitcast(F32R))
                    return t

                cwn0 = cwn_load(0, nc.sync)
                wnat0s = [wnat0_half(0)]
                # ACT queue (behind dw1): cwn 1/2, wnat0-h2
                cwns = [cwn0, cwn_load(1, nc.scalar), cwn_load(2, nc.scalar)]
                wnat0s.append(wnat0_half(1, nc.scalar))

                # ep_w0 -> W0fT / W0cT, batched evicts, two wnat0 halves
                W0fN = pro.tile([128, 5, 384], BF16, tag="w0fn", bufs=1)
                W0cN = pro.tile([128, 5, 384], BF16, tag="w0cn", bufs=1)
                def epw0_half(h):
                    wnat0 = wnat0s[h]
                    mchunks = list(enumerate(chunks_of(640)))[h * 3:(h + 1) * 3]
                    for mi, (ms, mw) in mchunks:
                        evict(W0fN[0:mw, mi, 0:384], wnat0[0:mw, mi % 3, 0:384])
                        evict(W0cN[0:mw, mi, 0:384], wnat0[0:mw, mi % 3, 384:768])
                    for ci in range(6):
                        cww = 128
                        pt = prps.tile([128, 4, 128], F32, tag="tp")
                        for k, (mi, (ms, mw)) in enumerate(mchunks):
                            nc.tensor.transpose(pt[0:cww, k, 0:mw],
                                                wnat0[0:mw, mi % 3, ci * 128:ci * 128 + cww],
                                                ident[0:mw, 0:mw])
                        ms0 = mchunks[0][1][0]
                        tw = sum(mw for _, (ms, mw) in mchunks)
                        span = pt[:, 0:len(mchunks), :].rearrange("p a b -> p (a b)")[0:cww, 0:tw]
                        if ci < 3:
                            evict(W0fT[0:cww, ci, ms0:ms0 + tw], span)
                        else:
                            evict(W0cT[0:cww, ci - 3, ms0:ms0 + tw], span)

                WdNf = [pro.tile([128, 3, 192], BF16, tag=f"WdNf{f}", name=f"WdNf{f}", bufs=1)
                        for f in range(2)]
                def wdt_mi(mi):
                    cwn = cwns[mi]
                    for d in OLD_TAPS:
                        ky, kx = 2 - d[0], 2 - d[1]
                        pt = prps.tile([128, 4, 128], F32, tag="tp")
                        for ci, (cs, cww) in enumerate(chunks_of(192)):
                            sap = _ap(cwn, 0, cs * 25 + ky * 5 + kx,
                                      [[192 * 25, 128], [25, cww]]).bitcast(F32)
                            nc.tensor.transpose(pt[0:cww, ci, 0:128], sap, ident[:])
                        dst = _ap(WdT[d], 0, mi * 128,
                                  [[2 * 384, 128], [384, 2], [1, 128]])
                        evict(dst, pt[:, 0:2, 0:128])
                    for f, d in enumerate(FRESH_TAPS):
                        ky, kx = 2 - d[0], 2 - d[1]
                        sap = _ap(cwn, 0, ky * 5 + kx,
                                  [[192 * 25, 128], [25, 192]]).bitcast(F32)
                        nc.vector.tensor_copy(WdNf[f][0:128, mi, 0:192], sap)

                wdt_mi(0)
                epw0_half(0)
                wdt_mi(1)
                wdt_mi(2)
                epw0_half(1)

                # Pool queue: ep_w1..5 (rotation stalls stay off HWDGE queues)
                WN = {li: pro.tile([128, cdiv(DIMS[li + 1], 128), DIMS[li]], BF16,
                                   tag=f"WN{li}", name=f"WN{li}", bufs=1)
                      for li in range(1, 5)}
                def load_and_transpose(li):
                    n_out, n_in = DIMS[li + 1], DIMS[li]
                    wnat = pro.tile([128, 5, 768], F32, tag="wnatS", name="wnatS", bufs=1)
                    for mi, (ms, mw) in enumerate(chunks_of(n_out)):
                        nc.gpsimd.dma_start(wnat[0:mw, mi, 0:n_in],
                                            di[f'ep_w{li}'].ap()[ms:ms + mw])
                    for ci, (cs, cww) in enumerate(chunks_of(n_in)):
                        mchunks = list(enumerate(chunks_of(n_out)))
                        for mb in range(cdiv(len(mchunks), 4)):
                            mcb = mchunks[mb * 4:(mb + 1) * 4]
                            pt = prps.tile([128, 4, 128], F32, tag="tp")
                            for k, (mi, (ms, mw)) in enumerate(mcb):
                                nc.tensor.transpose(pt[0:cww, k, 0:mw],
                                                    wnat[0:mw, mi, cs:cs + cww],
                                                    ident[0:mw, 0:mw])
                            ms0 = mcb[0][1][0]
                            tw = sum(mw for _, (ms, mw) in mcb)
                            span = pt[:, 0:len(mcb), :].rearrange("p a b -> p (a b)")[0:cww, 0:tw]
                            evict(WT[li][0:cww, ci, ms0:ms0 + tw], span, scale=0.99)
                    if li <= 4:
                        for mi, (ms, mw) in enumerate(chunks_of(n_out)):
                            evict(WN[li][0:mw, mi, 0:n_in], wnat[0:mw, mi, 0:n_in])
                for li in range(1, 6):
                    load_and_transpose(li)

                # GT[f] = (W0c @ Wd_tap)^T = WdN^T-compose (all bf16)
                for f in range(2):
                    for mc, (cs, cww) in enumerate(chunks_of(192)):
                        for nh in range(2):
                            gp = prps.tile([128, 512], F32, tag="dps")
                            for ki in range(3):
                                nc.tensor.matmul(gp[0:cww, 0:320],
                                                 WdNf[f][0:128, ki, cs:cs + cww],
                                                 W0cT[0:128, ki, nh * 320:(nh + 1) * 320],
                                                 start=(ki == 0), stop=(ki == 2),
                                                 skip_group_check=True)
                            evict(GT[f][0:cww, mc, nh * 320:(nh + 1) * 320],
                                  gp[0:cww, 0:320])

                # ---- relu-decomposition composites ----
                # PT[l] = 0.0099*(W_{l+1} W_l)^T  (WT tiles carry 0.99 each)
                for li in range(1, 5):
                    nN = DIMS[li + 2]
                    for m, (ms, mw) in enumerate(chunks_of(DIMS[li])):
                        gp = prps.tile([128, 512], F32, tag="dps")
                        kch = chunks_of(DIMS[li + 1])
                        for k, (ks, kw) in enumerate(kch):
                            nc.tensor.matmul(gp[0:mw, 0:nN],
                                             WN[li][0:kw, k, ms:ms + mw],
                                             WT[li + 1][0:kw, k, 0:nN],
                                             start=(k == 0), stop=(k == len(kch) - 1),
                                             skip_group_check=True)
                        evict(PT[li][0:mw, m, 0:nN], gp[0:mw, 0:nN],
                              scale=0.0099 / (0.99 * 0.99))

                # F1T/C1T = 0.01*(W1 W0f/c)^T  (WT[1] carries 0.99)
                for nat, dstT in ((W0fN, F1T), (W0cN, C1T)):
                    for m, (ms, mw) in enumerate(chunks_of(384)):
                        gp = prps.tile([128, 512], F32, tag="dps")
                        for k in range(5):
                            nc.tensor.matmul(gp[0:mw, 0:512],
                                             nat[0:128, k, ms:ms + mw],
                                             WT[1][0:128, k, 0:512],
                                             start=(k == 0), stop=(k == 4),
                                             skip_group_check=True)
                        evict(dstT[0:mw, m, 0:512], gp[0:mw, 0:512],
                              scale=0.01 / 0.99)

                # G1T[f] = (C1 @ Wd_tap)^T = WdN-compose with C1T (C1T has the 0.01)
                for f in range(2):
                    for mc, (cs, cww) in enumerate(chunks_of(192)):
                        gp = prps.tile([128, 512], F32, tag="dps")
                        for ki in range(3):
                            nc.tensor.matmul(gp[0:cww, 0:512],
                                             WdNf[f][0:128, ki, cs:cs + cww],
                                             C1T[0:128, ki, 0:512],
                                             start=(ki == 0), stop=(ki == 2),
                                             skip_group_check=True)
                        evict(G1T[f][0:cww, mc, 0:512], gp[0:cww, 0:512])

                # wbrow[l] = (0.01/0.99)*W_l b_{l-1} as a row (K=1 MM transposes)
                bcol = pro.tile([128, 6, 1], BF16, tag="bcol", bufs=1)
                wbc = pro.tile([128, 4, 1], BF16, tag="wbc", bufs=2)
                onne = pro.tile([1, 1], BF16, tag="onne", bufs=1)
                nc.vector.memset(onne[:], 1.0)
                for li in range(1, 6):
                    kch = chunks_of(DIMS[li])
                    bp = prps.tile([128, 6, 1], F32, tag="dps", name="bp")
                    for k, (ks, kw) in enumerate(kch):
                        nc.tensor.matmul(bp[0:kw, k, 0:1],
                                         brow[li - 1][0:1, ks:ks + kw],
                                         onne[0:1, 0:1],
                                         start=True, stop=True, skip_group_check=True)
                    nc.vector.tensor_copy(bcol[:, 0:len(kch), 0:1], bp[:, 0:len(kch), 0:1])
                    wbp = prps.tile([128, 4, 1], F32, tag="dps", name="wbp")
                    mch = chunks_of(DIMS[li + 1])
                    for m, (ms, mw) in enumerate(mch):
                        for k, (ks, kw) in enumerate(kch):
                            nc.tensor.matmul(wbp[0:mw, m, 0:1],
                                             WT[li][0:kw, k, ms:ms + mw],
                                             bcol[0:kw, k, 0:1],
                                             start=(k == 0), stop=(k == len(kch) - 1),
                                             skip_group_check=True)
                    nc.vector.tensor_copy(wbc[:, 0:len(mch), 0:1], wbp[:, 0:len(mch), 0:1])
                    rp = prps.tile([128, 512], F32, tag="dps", name="rp")
                    for m, (ms, mw) in enumerate(mch):
                        nc.tensor.matmul(rp[0:1, 0:mw], wbc[0:mw, m, 0:1],
                                         identB[0:mw, 0:mw],
                                         start=True, stop=True, skip_group_check=True)
                        evict(wbrow[li][0:1, ms:ms + mw], rp[0:1, 0:mw],
                              scale=0.01 / 0.99)

                # conv2 weights -> cw2T (cw2s via wnatS slot rotation)
                for mi in range(3):
                    cw2s = pro.tile([128, 288 * 9], F32, tag="wnatS",
                                    name=f"cw2s{mi}", bufs=1)
                    eng = nc.sync if mi == 0 else nc.gpsimd
                    eng.dma_start(
                        cw2s[:],
                        di['hs_cw2'].ap()[mi * 128:(mi + 1) * 128]
                        .rearrange("o c kh kw -> o (c kh kw)"))
                    nc.vector.memset(cw2T[32:64, mi, 2, :], 0.0)
                    nc.gpsimd.memset(cw2T[64:128, mi, 2, :], 0.0)
                    for si, (ss, sw) in enumerate(chunks_of(288)):
                        for kb in range(3):
                            ks = list(range(9))[kb * 4:(kb + 1) * 4]
                            if not ks:
                                continue
                            pt = prps.tile([128, 4, 128], F32, tag="tp")
                            for kk, k in enumerate(ks):
                                sap = _ap(cw2s, 0, ss * 9 + k, [[288 * 9, 128], [9, sw]])
                                nc.tensor.transpose(pt[0:sw, kk, 0:128], sap, ident[:])
                            span = pt[:, 0:len(ks), :].rearrange("p a b -> p (a b)")[0:sw, 0:len(ks) * 128]
                            evict(_ap(cw2T, (mi * 3 + si) * 9 * 128, ks[0] * 128,
                                      [[3 * 3 * 9 * 128, sw], [1, len(ks) * 128]]),
                                  span)

            # conv2 rows 0..1 upfront; rest streamed into the wavefront steps
            for mi in range(3):
                emit_conv2_unit(nc, pps, cw2T, m2, fm1, b_c2, mi, 0, 1, 0, 27)

            # ================= SEQUENTIAL PHASE =================
            # conv2 streaming: 2-row units (mi, rb), rows [2rb, 2rb+2)
            pf_units = [(mi, rb) for rb in range(1, 16) for mi in range(3)]
            pf_state = {"u": 0, "k": 0, "ps": None}

            def pfill(nmm):
                while nmm > 0 and pf_state["u"] < len(pf_units):
                    mi, rb = pf_units[pf_state["u"]]
                    take = min(nmm, 27 - pf_state["k"])
                    ps = emit_conv2_unit(nc, pps, cw2T, m2, fm1, b_c2, mi,
                                         2 * rb, 2 * rb + 2,
                                         pf_state["k"], pf_state["k"] + take,
                                         ps=pf_state["ps"])
                    pf_state["ps"] = ps
                    pf_state["k"] += take
                    nmm -= take
                    if pf_state["k"] == 27:
                        pf_state["u"] += 1
                        pf_state["k"] = 0
                        pf_state["ps"] = None

            def ydiag_ap(img, i0, j0, kw, c, B):
                """[kw, B] wavefront-diagonal AP into padded img tile chunk c."""
                off = (i0 + 2) * WP + (j0 + 2)
                return _ap(img, c * HP * WP, off, [[2 * HP * WP, kw], [DIAG, B]])

            def emit_seed2(pt, slot, brow_ap, mw, B):
                nc.tensor.matmul(pt[0:mw, slot, 0:B], brow_ap, ones[0:1, 0:B],
                                 start=True, stop=False, skip_group_check=True)

            def emit_old_ctx(t1):
                """ctx_b seed for step t1's ctx -> zc[t1%2][5:8]; return tap MM list."""
                s1 = t1 % 2
                i_lo, B, j_lo = step_geom(t1)
                ms_list = chunks_of(384)
                for m, (ms, mw) in enumerate(ms_list):
                    emit_seed2(zc[s1], 5 + m, ctxb[0:1, ms:ms + mw], mw, B)
                mms = []
                for ti, (dy, dx) in enumerate(OLD_TAPS):
                    for c, (cs, kw) in enumerate(chunks_of(192)):
                        for m, (ms, mw) in enumerate(ms_list):
                            mms.append((ti, dy, dx, c, cs, kw, m, ms, mw))
                return i_lo, B, j_lo, mms

            # prologue part of step 0's z0/ctx accumulation
            def emit_z0_pre(t1):
                """seeds + f-part + (later) ctx-part for z0 of step t1 -> zc[t1%2][0:5]"""
                s1 = t1 % 2
                i_lo, B, j_lo = step_geom(t1)
                for m, (ms, mw) in enumerate(chunks_of(640)):
                    emit_seed2(zc[s1], m, brow[0][0:1, ms:ms + mw], mw, B)
                for k in range(3):
                    for m, (ms, mw) in enumerate(chunks_of(640)):
                        rhs = _ap(fm1, k * NPIX, i_lo * W + j_lo,
                                  [[3 * NPIX, 128], [W - 3, B]])
                        nc.tensor.matmul(zc[s1][0:mw, m, 0:B],
                                         W0fT[0:128, k, ms:ms + mw], rhs,
                                         start=False, stop=False, skip_group_check=True)

            def emit_z0_ctx(t1):
                s1 = t1 % 2
                i_lo, B, j_lo = step_geom(t1)
                for k in range(3):
                    for m, (ms, mw) in enumerate(chunks_of(640)):
                        nc.tensor.matmul(zc[s1][0:mw, m, 0:B],
                                         W0cT[0:128, k, ms:ms + mw],
                                         Xc[0:128, k, 0:B],
                                         start=False, stop=False, skip_group_check=True)

            def emit_old_mms(t1, geom, mms):
                i_lo, B, j_lo = geom
                s1 = t1 % 2
                for (ti, dy, dx, c, cs, kw, m, ms, mw) in mms:
                    rhs = ydiag_ap(Yimg, i_lo - dy, j_lo - dx, kw, c, B)
                    last = (ti == len(OLD_TAPS) - 1) and (c == 1)
                    nc.tensor.matmul(zc[s1][0:mw, 5 + m, 0:B],
                                     WdT[OLD_TAPS[ti]][0:kw, c, ms:ms + mw], rhs,
                                     start=False, stop=last, skip_group_check=True)

            # --- step 0 pre-work (its sources are all zero borders) ---
            g0 = emit_old_ctx(0)
            emit_old_mms(0, (g0[0], g0[1], g0[2]), g0[3])
            i_lo0, B0, j_lo0 = step_geom(0)
            nc.vector.tensor_copy(Xc[:, 0:3, 0:B0], zc[0][:, 5:8, 0:B0])
            emit_z0_pre(0)
            emit_z0_ctx(0)

            KCHW = {li: chunks_of(DIMS[li]) for li in range(1, 6)}
            MCHW = {li: chunks_of(DIMS[li + 1]) for li in range(0, 6)}

            for t in range(nsteps):
                s = t % 2
                s1 = (t + 1) % 2
                i_lo, B, j_lo = step_geom(t)
                have_next = t + 1 < nsteps
                if have_next:
                    i_lo1, B1, j_lo1 = step_geom(t + 1)

                # ---- fresh taps -> z0 (critical) ----
                for m, (ms, mw) in enumerate(MCHW[0]):
                    for f in range(2):
                        dy, dx = FRESH_TAPS[f]
                        for c, (cs, kw) in enumerate(chunks_of(192)):
                            rhs = ydiag_ap(Yimg, i_lo - dy, j_lo - dx, kw, c, B)
                            nc.tensor.matmul(zc[s][0:mw, m, 0:B],
                                             GT[f][0:kw, c, ms:ms + mw], rhs,
                                             start=False,
                                             stop=(f == 1 and c == 1),
                                             skip_group_check=True)

                # z1 "early" contributions + all z seeds (run inside r0's hop)
                for li in range(1, 6):
                    for m, (ms, mw) in enumerate(MCHW[li]):
                        emit_seed2(zs, ZOFF[li] + m, brow[li][0:1, ms:ms + mw], mw, B)
                        nc.tensor.matmul(zs[0:mw, ZOFF[li] + m, 0:B],
                                         wbrow[li][0:1, ms:ms + mw], ones[0:1, 0:B],
                                         start=False, stop=False, skip_group_check=True)
                for m, (ms, mw) in enumerate(MCHW[1]):
                    for f in range(2):   # G1 fresh (0.01 level)
                        dy, dx = FRESH_TAPS[f]
                        for c, (cs, kw) in enumerate(chunks_of(192)):
                            rhs = ydiag_ap(Yimg, i_lo - dy, j_lo - dx, kw, c, B)
                            nc.tensor.matmul(zs[0:mw, ZOFF[1] + m, 0:B],
                                             G1T[f][0:kw, c, ms:ms + mw], rhs,
                                             start=False, stop=False,
                                             skip_group_check=True)
                    for k in range(3):   # C1 * ctx_old, F1 * f
                        nc.tensor.matmul(zs[0:mw, ZOFF[1] + m, 0:B],
                                         C1T[0:128, k, ms:ms + mw], Xc[0:128, k, 0:B],
                                         start=False, stop=False, skip_group_check=True)
                        rhs = _ap(fm1, k * NPIX, i_lo * W + j_lo,
                                  [[3 * NPIX, 128], [W - 3, B]])
                        nc.tensor.matmul(zs[0:mw, ZOFF[1] + m, 0:B],
                                         F1T[0:128, k, ms:ms + mw], rhs,
                                         start=False, stop=False, skip_group_check=True)

                # ---- r0 = relu(z0) (critical DVE) ----
                nc.vector.tensor_scalar_max(xs[1][:, 0:5, 0:B], zc[s][:, 0:5, 0:B], 0.0)

                # old-ctx for t+1 (fillers)
                old_mms = []
                if have_next:
                    g = emit_old_ctx(t + 1)
                    old_mms = g[3]
                    geom1 = (g[0], g[1], g[2])

                # ---- MLP layers 1..5 ----
                for li in range(1, 6):
                    # fillers in the gap before this layer's critical MMs
                    if li == 2 and have_next:
                        emit_old_mms(t + 1, geom1, old_mms[:33])
                    elif li == 3 and have_next:
                        emit_old_mms(t + 1, geom1, old_mms[33:])
                    elif li == 4 and have_next:
                        nc.scalar.activation(Xc[:, 0:3, 0:B1], zc[s1][:, 5:8, 0:B1],
                                             mybir.ActivationFunctionType.Copy)
                        emit_z0_pre(t + 1)
                        pfill(10)
                    elif li == 5 and have_next:
                        pfill(14)
                    # pair term P_{li-1} * r_{li-2} -> z_li (off critical path)
                    if li >= 2:
                        pl = li - 1
                        kch = chunks_of(DIMS[pl])
                        for m, (ms, mw) in enumerate(MCHW[li]):
                            for k, (ks, kw) in enumerate(kch):
                                nc.tensor.matmul(zs[0:mw, ZOFF[li] + m, 0:B],
                                                 PT[pl][0:kw, k, ms:ms + mw],
                                                 xs[pl][0:kw, k, 0:B],
                                                 start=False, stop=False,
                                                 skip_group_check=True)
                    # critical: 0.99*W_li * r_{li-1}
                    kch = KCHW[li]
                    for m, (ms, mw) in enumerate(MCHW[li]):
                        for k, (ks, kw) in enumerate(kch):
                            nc.tensor.matmul(zs[0:mw, ZOFF[li] + m, 0:B],
                                             WT[li][0:kw, k, ms:ms + mw],
                                             xs[li][0:kw, k, 0:B],
                                             start=False, stop=(k == len(kch) - 1),
                                             skip_group_check=True)
                    if li < 5:
                        nch = len(MCHW[li])
                        nc.vector.tensor_scalar_max(
                            xs[li + 1][:, 0:nch, 0:B],
                            zs[:, ZOFF[li]:ZOFF[li] + nch, 0:B], 0.0)

                # ---- Y = z5 + w_hat (critical DVE) ----
                off = (i_lo + 2) * WP + (j_lo + 2)
                ydst = _ap(Yimg, 0, off, [[2 * HP * WP, 128], [HP * WP, 2], [DIAG, B]])
                ywim = _ap(wimg, 0, i_lo * W + j_lo,
                           [[2 * NPIX, 128], [NPIX, 2], [W - 3, B]])
                nc.vector.tensor_tensor(ydst, zs[:, 12:14, 0:B], ywim,
                                        mybir.AluOpType.add)
                # late filler: ctx->z0 for t+1
                if have_next:
                    emit_z0_ctx(t + 1)

            # ================= EPILOGUE =================
            with tc.tile_pool(name="epi", bufs=1) as epi:
                Yimg32 = epi.tile([128, 2, NPIX], F32)
                src = _ap(Yimg, 0, 2 * WP + 2,
                          [[2 * HP * WP, 128], [HP * WP, 2], [WP, H], [1, W]])
                dst = _ap(Yimg32, 0, 0,
                          [[2 * NPIX, 128], [NPIX, 2], [W, H], [1, W]])
                nc.vector.tensor_copy(dst, src)
                ov = out.ap()[0]
                for ci, (cs, cw) in enumerate(chunks_of(192)):
                    nc.sync.dma_start(
                        ov[cs:cs + cw],
                        Yimg32[0:cw, ci, :].rearrange("p (h w) -> p h w", h=H))

    nc.compile()
    return nc


def emit_conv2_unit(nc, pps, cw2T, m2, fm1, b_c2, mi, r0, r1, k0, k1, ps=None):
    """Emit conv2 MMs [k0, k1) for out-chunk mi over rows [r0, r1);
    27 MMs per unit. MM index kk = k * 3 + si."""
    F32 = mybir.dt.float32
    nr = r1 - r0
    if ps is None:
        ps = pps.tile([128, 384], F32, tag="cps", name="cps")
    for kk in range(k0, k1):
        k, si = kk // 3, kk % 3
        ky, kx = k // 3, k % 3
        lhsT = _ap(cw2T, (mi * 3 + si) * 9 * 128, k * 128,
                   [[3 * 3 * 9 * 128, 128], [1, 128]])
        rhs = _ap(m2, si * 34 * 50, (ky + r0) * 50 + kx,
                  [[3 * 34 * 50, 128], [50, nr], [1, 48]])
        nc.tensor.matmul(ps[:, 0:nr * 48], lhsT, rhs,
                         start=(kk == 0), stop=(kk == 26), skip_group_check=True)
    if k1 == 27:
        nc.scalar.activation(fm1[:, mi, r0 * 48:r1 * 48], ps[:, 0:nr * 48],
                             mybir.ActivationFunctionType.Identity,
                             bias=b_c2[:, mi][:, None], alpha=0.0)
    return ps


_NC_CACHE = {}


def kernel(**inputs):
    from concourse.bass_utils import run_bass_kernel_spmd
    key = "full"
    if key not in _NC_CACHE:
        _NC_CACHE[key] = build()
    nc = _NC_CACHE[key]
    in_map = {k: np.ascontiguousarray(np.asarray(v, dtype=np.float32))
              for k, v in inputs.items()}
    res = run_bass_kernel_spmd(nc, [in_map] * 8, core_ids=list(range(8)))
    return res.results[0]['out']


if __name__ == "__main__":
    t = build(nsteps=int(sys.argv[1]) if len(sys.argv) > 1 else NSTEPS)
    print("build ok")
    from concourse.timeline_sim import TimelineSim
    est = TimelineSim(t).simulate()
    print(f"HW exec time: {est:.0f} ns")


# revision 43
# speedup vs baseline: 3.5216x; 3.4785x over previous
"""Trainium2 Bass kernel for nn_BEE_Bin2Symbol (hyper-decoder + masked-conv
autoregressive MLP decoder).

Architecture (v2, latency-oriented):
- Sequential phase runs all GEMMs in [C_out-on-partitions, B-pixels-free]
  orientation (weights stationary as lhsT, activations moving): matmul cost
  scales with B<=16, transposes disappear, and each layer's nonlinearity is a
  single DVE scalar_tensor_tensor op  max(0.01*z, z)  reading PSUM directly.
- 140 slope-3 anti-diagonal wavefronts. Per step the critical chain is
  fresh-tap MMs -> lrelu(z0) -> W1 MMs -> lrelu(z1) -> ... -> z5 -> Y-add.
  Everything else (old-tap ctx GEMMs, f-part, bias seeds, phase-P conv
  streaming) is issued into the PE gaps between the chain's engine hops.
- Fresh taps (age-1) use precomposed G = W0c @ Wd_tap so they feed z0
  directly; old taps accumulate a ctx PSUM a step ahead, evicted by the
  scalar engine (ACT) off the critical path.
- All sequential-phase weights/activations in bf16 (PSUM accumulate f32).
- Hyper-decoder (2 stride-2 deconvs + 3x3 conv) in f32r as phase-decomposed
  GEMMs; conv2's last 3 row-blocks stream into the first ~15 wavefront steps.
"""
import sys, os
sys.path.insert(0, "/opt/trn_rl_repo")

import numpy as np

import concourse.bass as bass
import concourse.bacc as bacc
import concourse.mybir as mybir
import concourse.tile as tile
from concourse.masks import make_identity

F32 = mybir.dt.float32
F32R = mybir.dt.float32r
BF16 = mybir.dt.bfloat16

H, W = 32, 48
HP, WP = H + 4, W + 4            # padded image 36 x 52
NPIX = H * W
NSTEPS = 3 * (H - 1) + (W - 1) + 1   # 140
DIAG = WP - 3                    # 49: wavefront-diagonal stride in padded img

# taps (dy, dx): source pixel = (i-dy, j-dx); ctx_w index (ky,kx) = (2-dy, 2-dx)
TAPS = [(2, 2), (2, 1), (2, 0), (2, -1), (2, -2),
        (1, 2), (1, 1), (1, 0), (1, -1), (1, -2),
        (0, 1), (0, 2)]
FRESH_TAPS = [(1, -2), (0, 1)]                # age-1 taps (need step t-1)
OLD_TAPS = [d for d in TAPS if d not in FRESH_TAPS]

DIMS = [768, 640, 512, 384, 320, 256, 192]   # MLP dims; layer l: DIMS[l]->DIMS[l+1]
# z1..z5 chunk offsets inside the shared zs psum tile [128, 14, 16]
ZOFF = {1: 0, 2: 4, 3: 7, 4: 10, 5: 12}


def cdiv(a, b):
    return (a + b - 1) // b


def chunks_of(n, c=128):
    return [(s, min(c, n - s)) for s in range(0, n, c)]


def _ap(tile_ap, slot_off, elem_off, plist):
    """Build a custom AP into a [128, S, F]-shaped sbuf/psum tile."""
    base = tile_ap[:]
    return bass.AP(base.tensor, base.offset + slot_off + elem_off, plist)


def step_geom(t):
    i_lo = max(0, cdiv(t - (W - 1), 3))
    i_hi = min(H - 1, t // 3)
    return i_lo, i_hi - i_lo + 1, t - 3 * i_lo


def build(nsteps=NSTEPS):
    nc = bacc.Bacc()

    # ---------------- DRAM I/O ----------------
    di = {}
    di['z_hat'] = nc.dram_tensor('z_hat', [1, 192, 8, 12], F32, kind="ExternalInput")
    di['w_hat'] = nc.dram_tensor('w_hat', [1, 192, 32, 48], F32, kind="ExternalInput")
    di['hs_dw0'] = nc.dram_tensor('hs_dw0', [192, 192, 5, 5], F32, kind="ExternalInput")
    di['hs_db0'] = nc.dram_tensor('hs_db0', [192], F32, kind="ExternalInput")
    di['hs_dw1'] = nc.dram_tensor('hs_dw1', [192, 288, 5, 5], F32, kind="ExternalInput")
    di['hs_db1'] = nc.dram_tensor('hs_db1', [288], F32, kind="ExternalInput")
    di['hs_cw2'] = nc.dram_tensor('hs_cw2', [384, 288, 3, 3], F32, kind="ExternalInput")
    di['hs_cb2'] = nc.dram_tensor('hs_cb2', [384], F32, kind="ExternalInput")
    di['ctx_w'] = nc.dram_tensor('ctx_w', [384, 192, 5, 5], F32, kind="ExternalInput")
    di['ctx_b'] = nc.dram_tensor('ctx_b', [384], F32, kind="ExternalInput")
    for li in range(6):
        di[f'ep_w{li}'] = nc.dram_tensor(f'ep_w{li}', [DIMS[li + 1], DIMS[li]], F32,
                                         kind="ExternalInput")
        di[f'ep_b{li}'] = nc.dram_tensor(f'ep_b{li}', [DIMS[li + 1]], F32,
                                         kind="ExternalInput")
    out = nc.dram_tensor('out', [1, 192, 32, 48], F32, kind="ExternalOutput")

    with tile.TileContext(nc) as tc:
        with tc.tile_pool(name="pp", bufs=1) as pp, \
             tc.tile_pool(name="pps", bufs=1, space="PSUM") as pps:

            ident = pp.tile([128, 128], F32)
            make_identity(nc, ident[:])

            # ---------- persistent state ----------
            Yimg = pp.tile([128, 2, HP * WP], BF16)   # decoded image (padded)
            nc.vector.memset(Yimg[:], 0.0)
            wimg = pp.tile([128, 2, NPIX], F32)       # w_hat residual (compact)
            fm1 = pp.tile([128, 3, NPIX], BF16)       # conv2 output [384, 1536]
            m2 = pp.tile([128, 3, 34 * 50], BF16)     # deconv1 out (padded 34x50)
            nc.gpsimd.memset(m2[:], 0.0)
            cw2T = pp.tile([128, 3, 3, 9 * 128], BF16)  # [cin, mi, si, k*128+o]

            # transposed weights (bf16)
            W0fT = pp.tile([128, 3, 640], BF16)
            W0cT = pp.tile([128, 3, 640], BF16)
            WT = {}
            for li in range(1, 6):
                WT[li] = pp.tile([128, cdiv(DIMS[li], 128), DIMS[li + 1]], BF16,
                                 tag=f"W{li}T", name=f"W{li}T")
            GT = [pp.tile([128, 2, 640], BF16, tag=f"GT{k}", name=f"GT{k}") for k in range(2)]
            # relu-decomposition composites (0.01*z linear flow)
            PT = {}   # PT[l] = 0.0099*(W_{l+1} W_l)^T  -> feeds z_{l+1} from r_{l-1}
            for li in range(1, 5):
                PT[li] = pp.tile([128, cdiv(DIMS[li], 128), DIMS[li + 2]], BF16,
                                 tag=f"PT{li}", name=f"PT{li}")
            F1T = pp.tile([128, 3, 512], BF16)   # 0.01*(W1 W0f)^T
            C1T = pp.tile([128, 3, 512], BF16)   # 0.01*(W1 W0c)^T
            G1T = [pp.tile([128, 2, 512], BF16, tag=f"G1T{k}", name=f"G1T{k}")
                   for k in range(2)]            # 0.01*(W1 G_tap)^T
            wbrow = [None] + [pp.tile([1, DIMS[li + 1]], BF16, tag=f"wb{li}", name=f"wb{li}")
                              for li in range(1, 6)]   # 0.01*W_l b_{l-1} rows
            identB = pp.tile([128, 128], BF16)
            make_identity(nc, identB[:])
            WdT = {}
            for d in OLD_TAPS:
                WdT[d] = pp.tile([128, 2, 384], BF16, tag=f"Wd{d[0]}_{d[1]}", name=f"Wd{d[0]}_{d[1]}")

            # bias rows (lhsT for K=1 seed matmuls)
            brow = [pp.tile([1, DIMS[li + 1]], BF16, tag=f"b{li}", name=f"b{li}") for li in range(6)]
            ctxb = pp.tile([1, 384], BF16)
            ones = pp.tile([1, 16], BF16)
            nc.vector.memset(ones[:], 1.0)

            # sequential-phase activations (single tiles; WAR handled by sems)
            xs = {li: pp.tile([128, cdiv(DIMS[li], 128), 16], BF16, tag=f"x{li}", name=f"x{li}")
                  for li in range(1, 6)}
            Xc = pp.tile([128, 3, 16], BF16)          # evicted old-ctx

            # persistent psum: z0+ctx ring (2 banks), mlp zs (1), conv stream (1)
            zc = [pps.tile([128, 8, 16], F32, tag=f"zc{s}", name=f"zc{s}") for s in range(2)]
            zs = pps.tile([128, 14, 16], F32, tag="zs")
            nc.vector.memset(zs[:], 0.0)
            nc.vector.memset(zc[0][:], 0.0)
            nc.vector.memset(zc[1][:], 0.0)

            # ============ PROLOGUE 1: hyper-decoder (DMA priority) ============
            proE_cm = tc.tile_pool(name="proE", bufs=1)
            proEarly = proE_cm.__enter__()
            with tc.tile_pool(name="proB", bufs=2) as pro, \
                 tc.tile_pool(name="prpsB", bufs=2, space="PSUM") as prps:

                # SP queue: deconv0 weights lead everything
                def dw_load(src_dram, ms, mw, eng):
                    pair = []
                    for ci, (cs, cww) in enumerate(chunks_of(192)):
                        dw = pro.tile([128, 128 * 25], F32R, tag="dw", name="dw", bufs=3)
                        eng.dma_start(
                            dw[0:cww, 0:mw * 25],
                            src_dram.ap()[cs:cs + cww, ms:ms + mw]
                            .rearrange("c o kh kw -> c (o kh kw)").bitcast(F32R))
                        pair.append(dw)
                    return pair
                dw0t = [dw_load(di['hs_dw0'], ms, mw, nc.sync)
                        for ms, mw in chunks_of(192)]

                # Pool/SWDGE queue: small loads (zp first - deconv0 input)
                zp = pro.tile([128, 2, 150], F32R, tag="zp", bufs=1)
                nc.vector.memset(zp[:].bitcast(F32), 0.0)
                zv = di['z_hat'].ap()[0]
                for ci, (s, cw) in enumerate(chunks_of(192)):
                    dst = _ap(zp, ci * 150, 14 + 1, [[2 * 150, cw], [14, 8], [1, 12]])
                    nc.gpsimd.dma_start(dst, zv[s:s + cw].bitcast(F32R))

                def load_bias_col(name, n):
                    nch = cdiv(n, 128)
                    t = pp.tile([128, nch], F32, tag=f"b_{name}", name=f"b_{name}")
                    nc.vector.memset(t[:], 0.0)
                    for ci, (s, w_) in enumerate(chunks_of(n)):
                        nc.gpsimd.dma_start(t[0:w_, ci:ci + 1], di[name][s:s + w_][:, None])
                    return t
                b_d0 = load_bias_col('hs_db0', 192)
                b_d1 = load_bias_col('hs_db1', 288)
                b_c2 = load_bias_col('hs_cb2', 384)

                # ctx_w staging in the long-lived early pool: its DMAs flow
                # underneath the deconv compute (no scope-memory WAR)
                cwns = []
                for cmi in range(3):
                    t = proEarly.tile([128, 192 * 25], F32R, tag="cwn",
                                      name=f"cwn{cmi}", bufs=2)
                    nc.gpsimd.dma_start(
                        t[:],
                        di['ctx_w'].ap()[cmi * 128:(cmi + 1) * 128]
                        .rearrange("o c kh kw -> o (c kh kw)").bitcast(F32R))
                    cwns.append(t)

                whv = di['w_hat'].ap()[0]
                for ci, (s, cw) in enumerate(chunks_of(192)):
                    nc.gpsimd.dma_start(
                        wimg[0:cw, ci, :].rearrange("p (h w) -> p h w", h=H),
                        whv[s:s + cw])

                m1 = proEarly.tile([128, 2, 18 * 26], F32R, tag="m1", bufs=1)
                nc.vector.memset(m1[:].bitcast(F32), 0.0)

                def deconv_chunk(inp_t, inp_hw, w_t, cin, mw, mi, out_t, bias_t):
                    hi, wi = inp_hw
                    ip_w = wi + 2
                    op_w = 2 * wi + 2
                    for py in range(2):
                        for px in range(2):
                            ps = prps.tile([128, 16 * 24], F32, tag="dps")
                            first = True
                            taps = [(u, v) for u in range(py, 5, 2) for v in range(px, 5, 2)]
                            for ti, (u, v) in enumerate(taps):
                                dy = (py + 2 - u) // 2
                                dx = (px + 2 - v) // 2
                                for ci, (cs, cww) in enumerate(chunks_of(cin)):
                                    lhsT = _ap(w_t[ci], 0, u * 5 + v,
                                               [[128 * 25, cww], [25, mw]])
                                    slot = (hi + 2) * ip_w if inp_t is not zp else 150
                                    rhs = _ap(inp_t, ci * slot,
                                              (1 + dy) * ip_w + (1 + dx),
                                              [[2 * slot, cww], [ip_w, hi], [1, wi]])
                                    last = (ti == len(taps) - 1) and (ci == len(chunks_of(cin)) - 1)
                                    nc.tensor.matmul(ps[0:mw, 0:hi * wi], lhsT, rhs,
                                                     start=first, stop=last,
                                                     skip_group_check=True)
                                    first = False
                            dst = _ap(out_t, mi * (2 * hi + 2) * op_w,
                                      (py + 1) * op_w + (px + 1),
                                      [[out_t.shape[1] * (2 * hi + 2) * op_w, mw],
                                       [2 * op_w, hi], [2, wi]])
                            nc.scalar.activation(
                                dst, ps[0:mw, 0:hi * wi].rearrange("p (a b) -> p a b", a=hi),
                                mybir.ActivationFunctionType.Lrelu,
                                bias=bias_t[0:mw, mi][:, None], alpha=0.01)

                # ACT queue: dw1 (own queue; SP carries dw0 + cwn)
                dw1t = [dw_load(di['hs_dw1'], ms, mw, nc.scalar)
                        for ms, mw in chunks_of(288)]

                # deconv0: z[192,8,12] -> m1[192,16,24]
                for mi, (ms, mw) in enumerate(chunks_of(192)):
                    deconv_chunk(zp, (8, 12), dw0t[mi], 192, mw, mi, m1, b_d0)

                # deconv1: m1[192,16,24] -> m2[288,32,48]
                for mi, (ms, mw) in enumerate(chunks_of(288)):
                    deconv_chunk(m1, (16, 24), dw1t[mi], 192, mw, mi, m2, b_d1)

            # ============ PROLOGUE 2: weight transposes ============
            with tc.tile_pool(name="pro", bufs=2) as pro, \
                 tc.tile_pool(name="prps", bufs=2, space="PSUM") as prps:

                tp_count = [0]
                def evict(dst_ap, src_ap, scale=None):
                    if scale is None and tp_count[0] % 2 == 0:
                        nc.vector.tensor_copy(dst_ap, src_ap)
                    elif scale is None:
                        nc.scalar.activation(dst_ap, src_ap,
                                             mybir.ActivationFunctionType.Copy)
                    elif tp_count[0] % 2 == 0:
                        nc.vector.tensor_scalar_mul(dst_ap, src_ap, float(scale))
                    else:
                        nc.scalar.activation(dst_ap, src_ap,
                                             mybir.ActivationFunctionType.Copy,
                                             scale=float(scale))
                    tp_count[0] += 1

                def wnat0_half(h, eng=None):
                    t = pro.tile([128, 5, 768], F32, tag="wnatS", name="wnat0",
                                 bufs=1)
                    e = eng or nc.sync
                    for mi, (ms, mw) in enumerate(chunks_of(640)):
                        if mi // 3 != h:
                            continue
                        e.dma_start(t[0:mw, mi % 3, 0:768],
                                    di['ep_w0'].ap()[ms:ms + mw])
                    return t


                wnat0s = [wnat0_half(0), wnat0_half(1, nc.scalar)]

                # brow loads (late; staging in this scope)
                def load_brow(dst, dram, n):
                    st = pro.tile([1, 640], F32, tag="brs", name="brs", bufs=1)
                    nc.gpsimd.dma_start(st[0:1, 0:n], dram.ap()[None, :])
                    nc.vector.tensor_copy(dst[0:1, 0:n], st[0:1, 0:n])
                for li in range(6):
                    load_brow(brow[li], di[f'ep_b{li}'], DIMS[li + 1])
                load_brow(ctxb, di['ctx_b'], 384)

                # ep_w0 -> W0fT / W0cT, batched evicts, two wnat0 halves
                W0fN = pro.tile([128, 5, 384], BF16, tag="w0fn", bufs=1)
                W0cN = pro.tile([128, 5, 384], BF16, tag="w0cn", bufs=1)
                def epw0_half(h):
                    wnat0 = wnat0s[h]
                    mchunks = list(enumerate(chunks_of(640)))[h * 3:(h + 1) * 3]
                    for mi, (ms, mw) in mchunks:
                        evict(W0fN[0:mw, mi, 0:384], wnat0[0:mw, mi % 3, 0:384])
                        evict(W0cN[0:mw, mi, 0:384], wnat0[0:mw, mi % 3, 384:768])
                    for ci in range(6):
                        cww = 128
                        pt = prps.tile([128, 4, 128], F32, tag="tp")
                        for k, (mi, (ms, mw)) in enumerate(mchunks):
                            nc.tensor.transpose(pt[0:cww, k, 0:mw],
                                                wnat0[0:mw, mi % 3, ci * 128:ci * 128 + cww],
                                                ident[0:mw, 0:mw])
                        ms0 = mchunks[0][1][0]
                        tw = sum(mw for _, (ms, mw) in mchunks)
                        span = pt[:, 0:len(mchunks), :].rearrange("p a b -> p (a b)")[0:cww, 0:tw]
                        if ci < 3:
                            evict(W0fT[0:cww, ci, ms0:ms0 + tw], span)
                        else:
                            evict(W0cT[0:cww, ci - 3, ms0:ms0 + tw], span)

                WdNf = [pro.tile([128, 3, 192], BF16, tag=f"WdNf{f}", name=f"WdNf{f}", bufs=1)
                        for f in range(2)]
                def wdt_mi(mi):
                    cwn = cwns[mi]
                    for d in OLD_TAPS:
                        ky, kx = 2 - d[0], 2 - d[1]
                        pt = prps.tile([128, 4, 128], F32, tag="tp")
                        for ci, (cs, cww) in enumerate(chunks_of(192)):
                            sap = _ap(cwn, 0, cs * 25 + ky * 5 + kx,
                                      [[192 * 25, 128], [25, cww]]).bitcast(F32)
                            nc.tensor.transpose(pt[0:cww, ci, 0:128], sap, ident[:])
                        dst = _ap(WdT[d], 0, mi * 128,
                                  [[2 * 384, 128], [384, 2], [1, 128]])
                        evict(dst, pt[:, 0:2, 0:128])
                    for f, d in enumerate(FRESH_TAPS):
                        ky, kx = 2 - d[0], 2 - d[1]
                        sap = _ap(cwn, 0, ky * 5 + kx,
                                  [[192 * 25, 128], [25, 192]]).bitcast(F32)
                        nc.vector.tensor_copy(WdNf[f][0:128, mi, 0:192], sap)

                wdt_mi(0)
                epw0_half(0)
                wdt_mi(1)
                wdt_mi(2)
                epw0_half(1)

                # Pool queue: ep_w1..5 (rotation stalls stay off HWDGE queues)
                WN = {li: pro.tile([128, cdiv(DIMS[li + 1], 128), DIMS[li]], BF16,
                                   tag=f"WN{li}", name=f"WN{li}", bufs=1)
                      for li in range(1, 5)}
                def load_and_transpose(li):
                    n_out, n_in = DIMS[li + 1], DIMS[li]
                    wnat = pro.tile([128, 5, 768], F32, tag="wnatS", name="wnatS", bufs=1)
                    for mi, (ms, mw) in enumerate(chunks_of(n_out)):
                        nc.gpsimd.dma_start(wnat[0:mw, mi, 0:n_in],
                                            di[f'ep_w{li}'].ap()[ms:ms + mw])
                    for ci, (cs, cww) in enumerate(chunks_of(n_in)):
                        mchunks = list(enumerate(chunks_of(n_out)))
                        for mb in range(cdiv(len(mchunks), 4)):
                            mcb = mchunks[mb * 4:(mb + 1) * 4]
                            pt = prps.tile([128, 4, 128], F32, tag="tp")
                            for k, (mi, (ms, mw)) in enumerate(mcb):
                                nc.tensor.transpose(pt[0:cww, k, 0:mw],
                                                    wnat[0:mw, mi, cs:cs + cww],
                                                    ident[0:mw, 0:mw])
                            ms0 = mcb[0][1][0]
                            tw = sum(mw for _, (ms, mw) in mcb)
                            span = pt[:, 0:len(mcb), :].rearrange("p a b -> p (a b)")[0:cww, 0:tw]
                            evict(WT[li][0:cww, ci, ms0:ms0 + tw], span, scale=0.99)
                    if li <= 4:
                        for mi, (ms, mw) in enumerate(chunks_of(n_out)):
                            evict(WN[li][0:mw, mi, 0:n_in], wnat[0:mw, mi, 0:n_in])
                for li in range(1, 6):
                    load_and_transpose(li)

                # GT[f] = (W0c @ Wd_tap)^T = WdN^T-compose (all bf16)
                for f in range(2):
                    for mc, (cs, cww) in enumerate(chunks_of(192)):
                        for nh in range(2):
                            gp = prps.tile([128, 512], F32, tag="dps")
                            for ki in range(3):
                                nc.tensor.matmul(gp[0:cww, 0:320],
                                                 WdNf[f][0:128, ki, cs:cs + cww],
                                                 W0cT[0:128, ki, nh * 320:(nh + 1) * 320],
                                                 start=(ki == 0), stop=(ki == 2),
                                                 skip_group_check=True)
                            evict(GT[f][0:cww, mc, nh * 320:(nh + 1) * 320],
                                  gp[0:cww, 0:320])

                # ---- relu-decomposition composites ----
                # PT[l] = 0.0099*(W_{l+1} W_l)^T  (WT tiles carry 0.99 each)
                for li in range(1, 5):
                    nN = DIMS[li + 2]
                    for m, (ms, mw) in enumerate(chunks_of(DIMS[li])):
                        gp = prps.tile([128, 512], F32, tag="dps")
                        kch = chunks_of(DIMS[li + 1])
                        for k, (ks, kw) in enumerate(kch):
                            nc.tensor.matmul(gp[0:mw, 0:nN],
                                             WN[li][0:kw, k, ms:ms + mw],
                                             WT[li + 1][0:kw, k, 0:nN],
                                             start=(k == 0), stop=(k == len(kch) - 1),
                                             skip_group_check=True)
                        evict(PT[li][0:mw, m, 0:nN], gp[0:mw, 0:nN],
                              scale=0.0099 / (0.99 * 0.99))

                # F1T/C1T = 0.01*(W1 W0f/c)^T  (WT[1] carries 0.99)
                for nat, dstT in ((W0fN, F1T), (W0cN, C1T)):
                    for m, (ms, mw) in enumerate(chunks_of(384)):
                        gp = prps.tile([128, 512], F32, tag="dps")
                        for k in range(5):
                            nc.tensor.matmul(gp[0:mw, 0:512],
                                             nat[0:128, k, ms:ms + mw],
                                             WT[1][0:128, k, 0:512],
                                             start=(k == 0), stop=(k == 4),
                                             skip_group_check=True)
                        evict(dstT[0:mw, m, 0:512], gp[0:mw, 0:512],
                              scale=0.01 / 0.99)

                # G1T[f] = (C1 @ Wd_tap)^T = WdN-compose with C1T (C1T has the 0.01)
                for f in range(2):
                    for mc, (cs, cww) in enumerate(chunks_of(192)):
                        gp = prps.tile([128, 512], F32, tag="dps")
                        for ki in range(3):
                            nc.tensor.matmul(gp[0:cww, 0:512],
                                             WdNf[f][0:128, ki, cs:cs + cww],
                                             C1T[0:128, ki, 0:512],
                                             start=(ki == 0), stop=(ki == 2),
                                             skip_group_check=True)
                        evict(G1T[f][0:cww, mc, 0:512], gp[0:cww, 0:512])

                # wbrow[l] = (0.01/0.99)*W_l b_{l-1} as a row (K=1 MM transposes)
                bcol = pro.tile([128, 6, 1], BF16, tag="bcol", bufs=1)
                wbc = pro.tile([128, 4, 1], BF16, tag="wbc", bufs=2)
                onne = pro.tile([1, 1], BF16, tag="onne", bufs=1)
                nc.vector.memset(onne[:], 1.0)
                for li in range(1, 6):
                    kch = chunks_of(DIMS[li])
                    bp = prps.tile([128, 6, 1], F32, tag="dps", name="bp")
                    for k, (ks, kw) in enumerate(kch):
                        nc.tensor.matmul(bp[0:kw, k, 0:1],
                                         brow[li - 1][0:1, ks:ks + kw],
                                         onne[0:1, 0:1],
                                         start=True, stop=True, skip_group_check=True)
                    nc.vector.tensor_copy(bcol[:, 0:len(kch), 0:1], bp[:, 0:len(kch), 0:1])
                    wbp = prps.tile([128, 4, 1], F32, tag="dps", name="wbp")
                    mch = chunks_of(DIMS[li + 1])
                    for m, (ms, mw) in enumerate(mch):
                        for k, (ks, kw) in enumerate(kch):
                            nc.tensor.matmul(wbp[0:mw, m, 0:1],
                                             WT[li][0:kw, k, ms:ms + mw],
                                             bcol[0:kw, k, 0:1],
                                             start=(k == 0), stop=(k == len(kch) - 1),
                                             skip_group_check=True)
                    nc.vector.tensor_copy(wbc[:, 0:len(mch), 0:1], wbp[:, 0:len(mch), 0:1])
                    rp = prps.tile([128, 512], F32, tag="dps", name="rp")
                    for m, (ms, mw) in enumerate(mch):
                        nc.tensor.matmul(rp[0:1, 0:mw], wbc[0:mw, m, 0:1],
                                         identB[0:mw, 0:mw],
                                         start=True, stop=True, skip_group_check=True)
                        evict(wbrow[li][0:1, ms:ms + mw], rp[0:1, 0:mw],
                              scale=0.01 / 0.99)

                # conv2 weights -> cw2T (cw2s via wnatS slot rotation)
                for mi in range(3):
                    cw2s = pro.tile([128, 288 * 9], F32, tag="wnatS",
                                    name=f"cw2s{mi}", bufs=1)
                    eng = nc.sync if mi == 0 else nc.gpsimd
                    eng.dma_start(
                        cw2s[:],
                        di['hs_cw2'].ap()[mi * 128:(mi + 1) * 128]
                        .rearrange("o c kh kw -> o (c kh kw)"))
                    nc.vector.memset(cw2T[32:64, mi, 2, :], 0.0)
                    nc.gpsimd.memset(cw2T[64:128, mi, 2, :], 0.0)
                    for si, (ss, sw) in enumerate(chunks_of(288)):
                        for kb in range(3):
                            ks = list(range(9))[kb * 4:(kb + 1) * 4]
                            if not ks:
                                continue
                            pt = prps.tile([128, 4, 128], F32, tag="tp")
                            for kk, k in enumerate(ks):
                                sap = _ap(cw2s, 0, ss * 9 + k, [[288 * 9, 128], [9, sw]])
                                nc.tensor.transpose(pt[0:sw, kk, 0:128], sap, ident[:])
                            span = pt[:, 0:len(ks), :].rearrange("p a b -> p (a b)")[0:sw, 0:len(ks) * 128]
                            evict(_ap(cw2T, (mi * 3 + si) * 9 * 128, ks[0] * 128,
                                      [[3 * 3 * 9 * 128, sw], [1, len(ks) * 128]]),
                                  span)

            proE_cm.__exit__(None, None, None)

            # conv2 rows 0..1 upfront; rest streamed into the wavefront steps
            for mi in range(3):
                emit_conv2_unit(nc, pps, cw2T, m2, fm1, b_c2, mi, 0, 1, 0, 27)

            # ================= SEQUENTIAL PHASE =================
            # conv2 streaming: 2-row units (mi, rb), rows [2rb, 2rb+2)
            pf_units = [(mi, rb) for rb in range(1, 16) for mi in range(3)]
            pf_state = {"u": 0, "k": 0, "ps": None}

            def pfill(nmm):
                while nmm > 0 and pf_state["u"] < len(pf_units):
                    mi, rb = pf_units[pf_state["u"]]
                    take = min(nmm, 27 - pf_state["k"])
                    ps = emit_conv2_unit(nc, pps, cw2T, m2, fm1, b_c2, mi,
                                         2 * rb, 2 * rb + 2,
                                         pf_state["k"], pf_state["k"] + take,
                                         ps=pf_state["ps"])
                    pf_state["ps"] = ps
                    pf_state["k"] += take
                    nmm -= take
                    if pf_state["k"] == 27:
                        pf_state["u"] += 1
                        pf_state["k"] = 0
                        pf_state["ps"] = None

            def ydiag_ap(img, i0, j0, kw, c, B):
                """[kw, B] wavefront-diagonal AP into padded img tile chunk c."""
                off = (i0 + 2) * WP + (j0 + 2)
                return _ap(img, c * HP * WP, off, [[2 * HP * WP, kw], [DIAG, B]])

            def emit_seed2(pt, slot, brow_ap, mw, B):
                nc.tensor.matmul(pt[0:mw, slot, 0:B], brow_ap, ones[0:1, 0:B],
                                 start=True, stop=False, skip_group_check=True)

            def emit_old_ctx(t1):
                """ctx_b seed for step t1's ctx -> zc[t1%2][5:8]; return tap MM list."""
                s1 = t1 % 2
                i_lo, B, j_lo = step_geom(t1)
                ms_list = chunks_of(384)
                for m, (ms, mw) in enumerate(ms_list):
                    emit_seed2(zc[s1], 5 + m, ctxb[0:1, ms:ms + mw], mw, B)
                mms = []
                for ti, (dy, dx) in enumerate(OLD_TAPS):
                    for c, (cs, kw) in enumerate(chunks_of(192)):
                        for m, (ms, mw) in enumerate(ms_list):
                            mms.append((ti, dy, dx, c, cs, kw, m, ms, mw))
                return i_lo, B, j_lo, mms

            # prologue part of step 0's z0/ctx accumulation
            def emit_z0_pre(t1):
                """seeds + f-part + (later) ctx-part for z0 of step t1 -> zc[t1%2][0:5]"""
                s1 = t1 % 2
                i_lo, B, j_lo = step_geom(t1)
                for m, (ms, mw) in enumerate(chunks_of(640)):
                    emit_seed2(zc[s1], m, brow[0][0:1, ms:ms + mw], mw, B)
                for k in range(3):
                    for m, (ms, mw) in enumerate(chunks_of(640)):
                        rhs = _ap(fm1, k * NPIX, i_lo * W + j_lo,
                                  [[3 * NPIX, 128], [W - 3, B]])
                        nc.tensor.matmul(zc[s1][0:mw, m, 0:B],
                                         W0fT[0:128, k, ms:ms + mw], rhs,
                                         start=False, stop=False, skip_group_check=True)

            def emit_z0_ctx(t1):
                s1 = t1 % 2
                i_lo, B, j_lo = step_geom(t1)
                for k in range(3):
                    for m, (ms, mw) in enumerate(chunks_of(640)):
                        nc.tensor.matmul(zc[s1][0:mw, m, 0:B],
                                         W0cT[0:128, k, ms:ms + mw],
                                         Xc[0:128, k, 0:B],
                                         start=False, stop=False, skip_group_check=True)

            def emit_old_mms(t1, geom, mms):
                i_lo, B, j_lo = geom
                s1 = t1 % 2
                for (ti, dy, dx, c, cs, kw, m, ms, mw) in mms:
                    rhs = ydiag_ap(Yimg, i_lo - dy, j_lo - dx, kw, c, B)
                    last = (ti == len(OLD_TAPS) - 1) and (c == 1)
                    nc.tensor.matmul(zc[s1][0:mw, 5 + m, 0:B],
                                     WdT[OLD_TAPS[ti]][0:kw, c, ms:ms + mw], rhs,
                                     start=False, stop=last, skip_group_check=True)

            # --- step 0 pre-work (its sources are all zero borders) ---
            g0 = emit_old_ctx(0)
            emit_old_mms(0, (g0[0], g0[1], g0[2]), g0[3])
            i_lo0, B0, j_lo0 = step_geom(0)
            nc.vector.tensor_copy(Xc[:, 0:3, 0:B0], zc[0][:, 5:8, 0:B0])
            emit_z0_pre(0)
            emit_z0_ctx(0)

            KCHW = {li: chunks_of(DIMS[li]) for li in range(1, 6)}
            MCHW = {li: chunks_of(DIMS[li + 1]) for li in range(0, 6)}

            for t in range(nsteps):
                s = t % 2
                s1 = (t + 1) % 2
                i_lo, B, j_lo = step_geom(t)
                have_next = t + 1 < nsteps
                if have_next:
                    i_lo1, B1, j_lo1 = step_geom(t + 1)

                # ---- fresh taps -> z0 (critical) ----
                for m, (ms, mw) in enumerate(MCHW[0]):
                    for f in range(2):
                        dy, dx = FRESH_TAPS[f]
                        for c, (cs, kw) in enumerate(chunks_of(192)):
                            rhs = ydiag_ap(Yimg, i_lo - dy, j_lo - dx, kw, c, B)
                            nc.tensor.matmul(zc[s][0:mw, m, 0:B],
                                             GT[f][0:kw, c, ms:ms + mw], rhs,
                                             start=False,
                                             stop=(f == 1 and c == 1),
                                             skip_group_check=True)

                # z1 "early" contributions + all z seeds (run inside r0's hop)
                for li in range(1, 6):
                    for m, (ms, mw) in enumerate(MCHW[li]):
                        emit_seed2(zs, ZOFF[li] + m, brow[li][0:1, ms:ms + mw], mw, B)
                        nc.tensor.matmul(zs[0:mw, ZOFF[li] + m, 0:B],
                                         wbrow[li][0:1, ms:ms + mw], ones[0:1, 0:B],
                                         start=False, stop=False, skip_group_check=True)
                for m, (ms, mw) in enumerate(MCHW[1]):
                    for f in range(2):   # G1 fresh (0.01 level)
                        dy, dx = FRESH_TAPS[f]
                        for c, (cs, kw) in enumerate(chunks_of(192)):
                            rhs = ydiag_ap(Yimg, i_lo - dy, j_lo - dx, kw, c, B)
                            nc.tensor.matmul(zs[0:mw, ZOFF[1] + m, 0:B],
                                             G1T[f][0:kw, c, ms:ms + mw], rhs,
                                             start=False, stop=False,
                                             skip_group_check=True)
                    for k in range(3):   # C1 * ctx_old, F1 * f
                        nc.tensor.matmul(zs[0:mw, ZOFF[1] + m, 0:B],
                                         C1T[0:128, k, ms:ms + mw], Xc[0:128, k, 0:B],
                                         start=False, stop=False, skip_group_check=True)
                        rhs = _ap(fm1, k * NPIX, i_lo * W + j_lo,
                                  [[3 * NPIX, 128], [W - 3, B]])
                        nc.tensor.matmul(zs[0:mw, ZOFF[1] + m, 0:B],
                                         F1T[0:128, k, ms:ms + mw], rhs,
                                         start=False, stop=False, skip_group_check=True)

                # ---- r0 = relu(z0) (critical DVE) ----
                nc.vector.tensor_scalar_max(xs[1][:, 0:5, 0:B], zc[s][:, 0:5, 0:B], 0.0)

                # old-ctx for t+1 (fillers)
                old_mms = []
                if have_next:
                    g = emit_old_ctx(t + 1)
                    old_mms = g[3]
                    geom1 = (g[0], g[1], g[2])

                # ---- MLP layers 1..5 ----
                for li in range(1, 6):
                    # fillers in the gap before this layer's critical MMs
                    if li == 2 and have_next:
                        emit_old_mms(t + 1, geom1, old_mms[:33])
                    elif li == 3 and have_next:
                        emit_old_mms(t + 1, geom1, old_mms[33:])
                    elif li == 4 and have_next:
                        nc.scalar.activation(Xc[:, 0:3, 0:B1], zc[s1][:, 5:8, 0:B1],
                                             mybir.ActivationFunctionType.Copy)
                        emit_z0_pre(t + 1)
                        pfill(10)
                    elif li == 5 and have_next:
                        pfill(14)
                    # pair term P_{li-1} * r_{li-2} -> z_li (off critical path)
                    if li >= 2:
                        pl = li - 1
                        kch = chunks_of(DIMS[pl])
                        for m, (ms, mw) in enumerate(MCHW[li]):
                            for k, (ks, kw) in enumerate(kch):
                                nc.tensor.matmul(zs[0:mw, ZOFF[li] + m, 0:B],
                                                 PT[pl][0:kw, k, ms:ms + mw],
                                                 xs[pl][0:kw, k, 0:B],
                                                 start=False, stop=False,
                                                 skip_group_check=True)
                    # critical: 0.99*W_li * r_{li-1}
                    kch = KCHW[li]
                    for m, (ms, mw) in enumerate(MCHW[li]):
                        for k, (ks, kw) in enumerate(kch):
                            nc.tensor.matmul(zs[0:mw, ZOFF[li] + m, 0:B],
                                             WT[li][0:kw, k, ms:ms + mw],
                                             xs[li][0:kw, k, 0:B],
                                             start=False, stop=(k == len(kch) - 1),
                                             skip_group_check=True)
                    if li < 5:
                        nch = len(MCHW[li])
                        nc.vector.tensor_scalar_max(
                            xs[li + 1][:, 0:nch, 0:B],
                            zs[:, ZOFF[li]:ZOFF[li] + nch, 0:B], 0.0)

                # ---- Y = z5 + w_hat (critical DVE) ----
                off = (i_lo + 2) * WP + (j_lo + 2)
                ydst = _ap(Yimg, 0, off, [[2 * HP * WP, 128], [HP * WP, 2], [DIAG, B]])
                ywim = _ap(wimg, 0, i_lo * W + j_lo,
                           [[2 * NPIX, 128], [NPIX, 2], [W - 3, B]])
                nc.vector.tensor_tensor(ydst, zs[:, 12:14, 0:B], ywim,
                                        mybir.AluOpType.add)
                # late filler: ctx->z0 for t+1
                if have_next:
                    emit_z0_ctx(t + 1)

            # ================= EPILOGUE =================
            with tc.tile_pool(name="epi", bufs=1) as epi:
                Yimg32 = epi.tile([128, 2, NPIX], F32)
                src = _ap(Yimg, 0, 2 * WP + 2,
                          [[2 * HP * WP, 128], [HP * WP, 2], [WP, H], [1, W]])
                dst = _ap(Yimg32, 0, 0,
                          [[2 * NPIX, 128], [NPIX, 2], [W, H], [1, W]])
                nc.vector.tensor_copy(dst, src)
                ov = out.ap()[0]
                for ci, (cs, cw) in enumerate(chunks_of(192)):
                    nc.sync.dma_start(
                        ov[cs:cs + cw],
                        Yimg32[0:cw, ci, :].rearrange("p (h w) -> p h w", h=H))

    nc.compile()
    return nc


def emit_conv2_unit(nc, pps, cw2T, m2, fm1, b_c2, mi, r0, r1, k0, k1, ps=None):
    """Emit conv2 MMs [k0, k1) for out-chunk mi over rows [r0, r1);
    27 MMs per unit. MM index kk = k * 3 + si."""
    F32 = mybir.dt.float32
    nr = r1 - r0
    if ps is None:
        ps = pps.tile([128, 384], F32, tag="cps", name="cps")
    for kk in range(k0, k1):
        k, si = kk // 3, kk % 3
        ky, kx = k // 3, k % 3
        lhsT = _ap(cw2T, (mi * 3 + si) * 9 * 128, k * 128,
                   [[3 * 3 * 9 * 128, 128], [1, 128]])
        rhs = _ap(m2, si * 34 * 50, (ky + r0) * 50 + kx,
                  [[3 * 34 * 50, 128], [50, nr], [1, 48]])
        nc.tensor.matmul(ps[:, 0:nr * 48], lhsT, rhs,
                         start=(kk == 0), stop=(kk == 26), skip_group_check=True)
    if k1 == 27:
        nc.scalar.activation(fm1[:, mi, r0 * 48:r1 * 48], ps[:, 0:nr * 48],
                             mybir.ActivationFunctionType.Identity,
                             bias=b_c2[:, mi][:, None], alpha=0.0)
    return ps


_NC_CACHE = {}


def kernel(**inputs):
    from concourse.bass_utils import run_bass_kernel_spmd
    key = "full"
    if key not in _NC_CACHE:
        _NC_CACHE[key] = build()
    nc = _NC_CACHE[key]
    in_map = {k: np.ascontiguousarray(np.asarray(v, dtype=np.float32))
              for k, v in inputs.items()}
    res = run_bass_kernel_spmd(nc, [in_map] * 8, core_ids=list(range(8)))
    return res.results[0]['out']


if __name__ == "__main__":
    t = build(nsteps=int(sys.argv[1]) if len(sys.argv) > 1 else NSTEPS)
    print("build ok")
    from concourse.timeline_sim import TimelineSim
    est = TimelineSim(t).simulate()
    print(f"HW exec time: {est:.0f} ns")


# revision 44
# speedup vs baseline: 4.0238x; 1.1426x over previous
"""Trainium2 Bass kernel for nn_BEE_Bin2Symbol (hyper-decoder + masked-conv
autoregressive MLP decoder).

Architecture (v2, latency-oriented):
- Sequential phase runs all GEMMs in [C_out-on-partitions, B-pixels-free]
  orientation (weights stationary as lhsT, activations moving): matmul cost
  scales with B<=16, transposes disappear, and each layer's nonlinearity is a
  single DVE scalar_tensor_tensor op  max(0.01*z, z)  reading PSUM directly.
- 140 slope-3 anti-diagonal wavefronts. Per step the critical chain is
  fresh-tap MMs -> lrelu(z0) -> W1 MMs -> lrelu(z1) -> ... -> z5 -> Y-add.
  Everything else (old-tap ctx GEMMs, f-part, bias seeds, phase-P conv
  streaming) is issued into the PE gaps between the chain's engine hops.
- Fresh taps (age-1) use precomposed G = W0c @ Wd_tap so they feed z0
  directly; old taps accumulate a ctx PSUM a step ahead, evicted by the
  scalar engine (ACT) off the critical path.
- All sequential-phase weights/activations in bf16 (PSUM accumulate f32).
- Hyper-decoder (2 stride-2 deconvs + 3x3 conv) in f32r as phase-decomposed
  GEMMs; conv2's last 3 row-blocks stream into the first ~15 wavefront steps.
"""
import sys, os
sys.path.insert(0, "/opt/trn_rl_repo")

import numpy as np

import concourse.bass as bass
import concourse.bacc as bacc
import concourse.mybir as mybir
import concourse.tile as tile
from concourse.masks import make_identity

F32 = mybir.dt.float32
F32R = mybir.dt.float32r
BF16 = mybir.dt.bfloat16

H, W = 32, 48
HP, WP = H + 4, W + 4            # padded image 36 x 52
NPIX = H * W
NSTEPS = 3 * (H - 1) + (W - 1) + 1   # 140
DIAG = WP - 3                    # 49: wavefront-diagonal stride in padded img

# taps (dy, dx): source pixel = (i-dy, j-dx); ctx_w index (ky,kx) = (2-dy, 2-dx)
TAPS = [(2, 2), (2, 1), (2, 0), (2, -1), (2, -2),
        (1, 2), (1, 1), (1, 0), (1, -1), (1, -2),
        (0, 1), (0, 2)]
FRESH_TAPS = [(1, -2), (0, 1)]                # age-1 taps (need step t-1)
OLD_TAPS = [d for d in TAPS if d not in FRESH_TAPS]

DIMS = [768, 640, 512, 384, 320, 256, 192]   # MLP dims; layer l: DIMS[l]->DIMS[l+1]
# z1..z5 chunk offsets inside the shared zs psum tile [128, 14, 16]
ZOFF = {1: 0, 2: 4, 3: 7, 4: 10, 5: 12}


def cdiv(a, b):
    return (a + b - 1) // b


def chunks_of(n, c=128):
    return [(s, min(c, n - s)) for s in range(0, n, c)]


def _ap(tile_ap, slot_off, elem_off, plist):
    """Build a custom AP into a [128, S, F]-shaped sbuf/psum tile."""
    base = tile_ap[:]
    return bass.AP(base.tensor, base.offset + slot_off + elem_off, plist)


def step_geom(t):
    i_lo = max(0, cdiv(t - (W - 1), 3))
    i_hi = min(H - 1, t // 3)
    return i_lo, i_hi - i_lo + 1, t - 3 * i_lo


def build(nsteps=NSTEPS):
    nc = bacc.Bacc()

    # ---------------- DRAM I/O ----------------
    di = {}
    di['z_hat'] = nc.dram_tensor('z_hat', [1, 192, 8, 12], F32, kind="ExternalInput")
    di['w_hat'] = nc.dram_tensor('w_hat', [1, 192, 32, 48], F32, kind="ExternalInput")
    di['hs_dw0'] = nc.dram_tensor('hs_dw0', [192, 192, 5, 5], F32, kind="ExternalInput")
    di['hs_db0'] = nc.dram_tensor('hs_db0', [192], F32, kind="ExternalInput")
    di['hs_dw1'] = nc.dram_tensor('hs_dw1', [192, 288, 5, 5], F32, kind="ExternalInput")
    di['hs_db1'] = nc.dram_tensor('hs_db1', [288], F32, kind="ExternalInput")
    di['hs_cw2'] = nc.dram_tensor('hs_cw2', [384, 288, 3, 3], F32, kind="ExternalInput")
    di['hs_cb2'] = nc.dram_tensor('hs_cb2', [384], F32, kind="ExternalInput")
    di['ctx_w'] = nc.dram_tensor('ctx_w', [384, 192, 5, 5], F32, kind="ExternalInput")
    di['ctx_b'] = nc.dram_tensor('ctx_b', [384], F32, kind="ExternalInput")
    for li in range(6):
        di[f'ep_w{li}'] = nc.dram_tensor(f'ep_w{li}', [DIMS[li + 1], DIMS[li]], F32,
                                         kind="ExternalInput")
        di[f'ep_b{li}'] = nc.dram_tensor(f'ep_b{li}', [DIMS[li + 1]], F32,
                                         kind="ExternalInput")
    out = nc.dram_tensor('out', [1, 192, 32, 48], F32, kind="ExternalOutput")

    with tile.TileContext(nc) as tc:
        with tc.tile_pool(name="pp", bufs=1) as pp, \
             tc.tile_pool(name="pps", bufs=1, space="PSUM") as pps:

            ident = pp.tile([128, 128], F32)
            make_identity(nc, ident[:])

            # ---------- persistent state ----------
            Yimg = pp.tile([128, 2, HP * WP], BF16)   # decoded image (padded)
            nc.vector.memset(Yimg[:], 0.0)
            wimg = pp.tile([128, 2, NPIX], F32)       # w_hat residual (compact)
            fm1 = pp.tile([128, 3, NPIX], BF16)       # conv2 output [384, 1536]
            m2 = pp.tile([128, 3, 34 * 50], BF16)     # deconv1 out (padded 34x50)
            nc.gpsimd.memset(m2[:], 0.0)
            cw2T = pp.tile([128, 3, 3, 9 * 128], BF16)  # [cin, mi, si, k*128+o]

            # transposed weights (bf16)
            W0fT = pp.tile([128, 3, 640], BF16)
            W0cT = pp.tile([128, 3, 640], BF16)
            WT = {}
            for li in range(1, 6):
                WT[li] = pp.tile([128, cdiv(DIMS[li], 128), DIMS[li + 1]], BF16,
                                 tag=f"W{li}T", name=f"W{li}T")
            GT = [pp.tile([128, 2, 640], BF16, tag=f"GT{k}", name=f"GT{k}") for k in range(2)]
            # relu-decomposition composites (0.01*z linear flow)
            PT = {}   # PT[l] = 0.0099*(W_{l+1} W_l)^T  -> feeds z_{l+1} from r_{l-1}
            for li in range(1, 5):
                PT[li] = pp.tile([128, cdiv(DIMS[li], 128), DIMS[li + 2]], BF16,
                                 tag=f"PT{li}", name=f"PT{li}")
            F1T = pp.tile([128, 3, 512], BF16)   # 0.01*(W1 W0f)^T
            C1T = pp.tile([128, 3, 512], BF16)   # 0.01*(W1 W0c)^T
            G1T = [pp.tile([128, 2, 512], BF16, tag=f"G1T{k}", name=f"G1T{k}")
                   for k in range(2)]            # 0.01*(W1 G_tap)^T
            wbrow = [None] + [pp.tile([1, DIMS[li + 1]], BF16, tag=f"wb{li}", name=f"wb{li}")
                              for li in range(1, 6)]   # 0.01*W_l b_{l-1} rows
            identB = pp.tile([128, 128], BF16)
            make_identity(nc, identB[:])
            WdT = {}
            for d in OLD_TAPS:
                WdT[d] = pp.tile([128, 2, 384], BF16, tag=f"Wd{d[0]}_{d[1]}", name=f"Wd{d[0]}_{d[1]}")

            # bias rows (lhsT for K=1 seed matmuls)
            brow = [pp.tile([1, DIMS[li + 1]], BF16, tag=f"b{li}", name=f"b{li}") for li in range(6)]
            ctxb = pp.tile([1, 384], BF16)
            ones = pp.tile([1, 16], BF16)
            nc.vector.memset(ones[:], 1.0)

            # sequential-phase activations (single tiles; WAR handled by sems)
            xs = {li: pp.tile([128, cdiv(DIMS[li], 128), 16], BF16, tag=f"x{li}", name=f"x{li}")
                  for li in range(1, 6)}
            Xc = pp.tile([128, 3, 16], BF16)          # evicted old-ctx

            # persistent psum: z0+ctx ring (2 banks), mlp zs (1), conv stream (1)
            zc = [pps.tile([128, 8, 16], F32, tag=f"zc{s}", name=f"zc{s}") for s in range(2)]
            zs = pps.tile([128, 14, 16], F32, tag="zs")
            nc.vector.memset(zs[:], 0.0)
            nc.vector.memset(zc[0][:], 0.0)
            nc.vector.memset(zc[1][:], 0.0)

            # ============ PROLOGUE 1: hyper-decoder (DMA priority) ============
            proE_cm = tc.tile_pool(name="proE", bufs=1)
            proEarly = proE_cm.__enter__()
            with tc.tile_pool(name="proB", bufs=2) as pro, \
                 tc.tile_pool(name="prpsB", bufs=2, space="PSUM") as prps:

                # SP queue: deconv0 weights lead everything
                def dw_load(src_dram, ms, mw, eng):
                    pair = []
                    for ci, (cs, cww) in enumerate(chunks_of(192)):
                        dw = pro.tile([128, 128 * 25], F32R, tag="dw", name="dw", bufs=4)
                        eng.dma_start(
                            dw[0:cww, 0:mw * 25],
                            src_dram.ap()[cs:cs + cww, ms:ms + mw]
                            .rearrange("c o kh kw -> c (o kh kw)").bitcast(F32R))
                        pair.append(dw)
                    return pair
                dw0t = [dw_load(di['hs_dw0'], ms, mw, nc.sync)
                        for ms, mw in chunks_of(192)]

                # Pool/SWDGE queue: small loads (zp first - deconv0 input)
                zp = pro.tile([128, 2, 150], F32R, tag="zp", bufs=1)
                nc.vector.memset(zp[:].bitcast(F32), 0.0)
                zv = di['z_hat'].ap()[0]
                for ci, (s, cw) in enumerate(chunks_of(192)):
                    dst = _ap(zp, ci * 150, 14 + 1, [[2 * 150, cw], [14, 8], [1, 12]])
                    nc.gpsimd.dma_start(dst, zv[s:s + cw].bitcast(F32R))

                def load_bias_col(name, n):
                    nch = cdiv(n, 128)
                    t = pp.tile([128, nch], F32, tag=f"b_{name}", name=f"b_{name}")
                    nc.vector.memset(t[:], 0.0)
                    for ci, (s, w_) in enumerate(chunks_of(n)):
                        nc.gpsimd.dma_start(t[0:w_, ci:ci + 1], di[name][s:s + w_][:, None])
                    return t
                b_d0 = load_bias_col('hs_db0', 192)
                b_d1 = load_bias_col('hs_db1', 288)
                b_c2 = load_bias_col('hs_cb2', 384)

                # ctx_w staging in the long-lived early pool: its DMAs flow
                # underneath the deconv compute (no scope-memory WAR)
                cwns = []
                for cmi in range(3):
                    t = proEarly.tile([128, 192 * 25], F32R, tag="cwn",
                                      name=f"cwn{cmi}", bufs=1)
                    nc.gpsimd.dma_start(
                        t[:],
                        di['ctx_w'].ap()[cmi * 128:(cmi + 1) * 128]
                        .rearrange("o c kh kw -> o (c kh kw)").bitcast(F32R))
                    cwns.append(t)

                whv = di['w_hat'].ap()[0]
                for ci, (s, cw) in enumerate(chunks_of(192)):
                    nc.gpsimd.dma_start(
                        wimg[0:cw, ci, :].rearrange("p (h w) -> p h w", h=H),
                        whv[s:s + cw])

                m1 = pro.tile([128, 2, 18 * 26], F32R, tag="m1", bufs=1)
                nc.vector.memset(m1[:].bitcast(F32), 0.0)

                def deconv_chunk(inp_t, inp_hw, w_t, cin, mw, mi, out_t, bias_t):
                    hi, wi = inp_hw
                    ip_w = wi + 2
                    op_w = 2 * wi + 2
                    for py in range(2):
                        for px in range(2):
                            ps = prps.tile([128, 16 * 24], F32, tag="dps")
                            first = True
                            taps = [(u, v) for u in range(py, 5, 2) for v in range(px, 5, 2)]
                            for ti, (u, v) in enumerate(taps):
                                dy = (py + 2 - u) // 2
                                dx = (px + 2 - v) // 2
                                for ci, (cs, cww) in enumerate(chunks_of(cin)):
                                    lhsT = _ap(w_t[ci], 0, u * 5 + v,
                                               [[128 * 25, cww], [25, mw]])
                                    slot = (hi + 2) * ip_w if inp_t is not zp else 150
                                    rhs = _ap(inp_t, ci * slot,
                                              (1 + dy) * ip_w + (1 + dx),
                                              [[2 * slot, cww], [ip_w, hi], [1, wi]])
                                    last = (ti == len(taps) - 1) and (ci == len(chunks_of(cin)) - 1)
                                    nc.tensor.matmul(ps[0:mw, 0:hi * wi], lhsT, rhs,
                                                     start=first, stop=last,
                                                     skip_group_check=True)
                                    first = False
                            dst = _ap(out_t, mi * (2 * hi + 2) * op_w,
                                      (py + 1) * op_w + (px + 1),
                                      [[out_t.shape[1] * (2 * hi + 2) * op_w, mw],
                                       [2 * op_w, hi], [2, wi]])
                            nc.scalar.activation(
                                dst, ps[0:mw, 0:hi * wi].rearrange("p (a b) -> p a b", a=hi),
                                mybir.ActivationFunctionType.Lrelu,
                                bias=bias_t[0:mw, mi][:, None], alpha=0.01)

                # ACT queue: dw1 (own queue; SP carries dw0 + cwn)
                dw1t = [dw_load(di['hs_dw1'], ms, mw, nc.scalar)
                        for ms, mw in chunks_of(288)]

                # deconv0: z[192,8,12] -> m1[192,16,24]
                for mi, (ms, mw) in enumerate(chunks_of(192)):
                    deconv_chunk(zp, (8, 12), dw0t[mi], 192, mw, mi, m1, b_d0)

                # deconv1: m1[192,16,24] -> m2[288,32,48]
                for mi, (ms, mw) in enumerate(chunks_of(288)):
                    deconv_chunk(m1, (16, 24), dw1t[mi], 192, mw, mi, m2, b_d1)

            # ============ PROLOGUE 2: weight transposes ============
            with tc.tile_pool(name="pro", bufs=2) as pro, \
                 tc.tile_pool(name="prps", bufs=2, space="PSUM") as prps:

                tp_count = [0]
                def evict(dst_ap, src_ap, scale=None):
                    if scale is None and tp_count[0] % 2 == 0:
                        nc.vector.tensor_copy(dst_ap, src_ap)
                    elif scale is None:
                        nc.scalar.activation(dst_ap, src_ap,
                                             mybir.ActivationFunctionType.Copy)
                    elif tp_count[0] % 2 == 0:
                        nc.vector.tensor_scalar_mul(dst_ap, src_ap, float(scale))
                    else:
                        nc.scalar.activation(dst_ap, src_ap,
                                             mybir.ActivationFunctionType.Copy,
                                             scale=float(scale))
                    tp_count[0] += 1

                def wnat0_half(h, eng=None):
                    t = pro.tile([128, 5, 768], F32, tag="wnatS", name="wnat0",
                                 bufs=1)
                    e = eng or nc.sync
                    for mi, (ms, mw) in enumerate(chunks_of(640)):
                        if mi // 3 != h:
                            continue
                        e.dma_start(t[0:mw, mi % 3, 0:768],
                                    di['ep_w0'].ap()[ms:ms + mw])
                    return t


                wnat0s = [wnat0_half(0), wnat0_half(1, nc.scalar)]

                # brow loads (late; staging in this scope)
                def load_brow(dst, dram, n):
                    st = pro.tile([1, 640], F32, tag="brs", name="brs", bufs=1)
                    nc.gpsimd.dma_start(st[0:1, 0:n], dram.ap()[None, :])
                    nc.vector.tensor_copy(dst[0:1, 0:n], st[0:1, 0:n])
                for li in range(6):
                    load_brow(brow[li], di[f'ep_b{li}'], DIMS[li + 1])
                load_brow(ctxb, di['ctx_b'], 384)

                # ep_w0 -> W0fT / W0cT, batched evicts, two wnat0 halves
                W0fN = pro.tile([128, 5, 384], BF16, tag="w0fn", bufs=1)
                W0cN = pro.tile([128, 5, 384], BF16, tag="w0cn", bufs=1)
                def epw0_half(h):
                    wnat0 = wnat0s[h]
                    mchunks = list(enumerate(chunks_of(640)))[h * 3:(h + 1) * 3]
                    for mi, (ms, mw) in mchunks:
                        evict(W0fN[0:mw, mi, 0:384], wnat0[0:mw, mi % 3, 0:384])
                        evict(W0cN[0:mw, mi, 0:384], wnat0[0:mw, mi % 3, 384:768])
                    for ci in range(6):
                        cww = 128
                        pt = prps.tile([128, 4, 128], F32, tag="tp")
                        for k, (mi, (ms, mw)) in enumerate(mchunks):
                            nc.tensor.transpose(pt[0:cww, k, 0:mw],
                                                wnat0[0:mw, mi % 3, ci * 128:ci * 128 + cww],
                                                ident[0:mw, 0:mw])
                        ms0 = mchunks[0][1][0]
                        tw = sum(mw for _, (ms, mw) in mchunks)
                        span = pt[:, 0:len(mchunks), :].rearrange("p a b -> p (a b)")[0:cww, 0:tw]
                        if ci < 3:
                            evict(W0fT[0:cww, ci, ms0:ms0 + tw], span)
                        else:
                            evict(W0cT[0:cww, ci - 3, ms0:ms0 + tw], span)

                WdNf = [pro.tile([128, 3, 192], BF16, tag=f"WdNf{f}", name=f"WdNf{f}", bufs=1)
                        for f in range(2)]
                def wdt_mi(mi):
                    cwn = cwns[mi]
                    for d in OLD_TAPS:
                        ky, kx = 2 - d[0], 2 - d[1]
                        pt = prps.tile([128, 4, 128], F32, tag="tp")
                        for ci, (cs, cww) in enumerate(chunks_of(192)):
                            sap = _ap(cwn, 0, cs * 25 + ky * 5 + kx,
                                      [[192 * 25, 128], [25, cww]]).bitcast(F32)
                            nc.tensor.transpose(pt[0:cww, ci, 0:128], sap, ident[:])
                        dst = _ap(WdT[d], 0, mi * 128,
                                  [[2 * 384, 128], [384, 2], [1, 128]])
                        evict(dst, pt[:, 0:2, 0:128])
                    for f, d in enumerate(FRESH_TAPS):
                        ky, kx = 2 - d[0], 2 - d[1]
                        sap = _ap(cwn, 0, ky * 5 + kx,
                                  [[192 * 25, 128], [25, 192]]).bitcast(F32)
                        nc.vector.tensor_copy(WdNf[f][0:128, mi, 0:192], sap)

                wdt_mi(0)
                epw0_half(0)
                wdt_mi(1)
                wdt_mi(2)
                epw0_half(1)

                # Pool queue: ep_w1..5 (rotation stalls stay off HWDGE queues)
                WN = {li: pro.tile([128, cdiv(DIMS[li + 1], 128), DIMS[li]], BF16,
                                   tag=f"WN{li}", name=f"WN{li}", bufs=1)
                      for li in range(1, 5)}
                def load_and_transpose(li):
                    n_out, n_in = DIMS[li + 1], DIMS[li]
                    wnat = pro.tile([128, 5, 768], F32, tag="wnatS", name="wnatS", bufs=1)
                    for mi, (ms, mw) in enumerate(chunks_of(n_out)):
                        nc.gpsimd.dma_start(wnat[0:mw, mi, 0:n_in],
                                            di[f'ep_w{li}'].ap()[ms:ms + mw])
                    for ci, (cs, cww) in enumerate(chunks_of(n_in)):
                        mchunks = list(enumerate(chunks_of(n_out)))
                        for mb in range(cdiv(len(mchunks), 4)):
                            mcb = mchunks[mb * 4:(mb + 1) * 4]
                            pt = prps.tile([128, 4, 128], F32, tag="tp")
                            for k, (mi, (ms, mw)) in enumerate(mcb):
                                nc.tensor.transpose(pt[0:cww, k, 0:mw],
                                                    wnat[0:mw, mi, cs:cs + cww],
                                                    ident[0:mw, 0:mw])
                            ms0 = mcb[0][1][0]
                            tw = sum(mw for _, (ms, mw) in mcb)
                            span = pt[:, 0:len(mcb), :].rearrange("p a b -> p (a b)")[0:cww, 0:tw]
                            evict(WT[li][0:cww, ci, ms0:ms0 + tw], span, scale=0.99)
                    if li <= 4:
                        for mi, (ms, mw) in enumerate(chunks_of(n_out)):
                            evict(WN[li][0:mw, mi, 0:n_in], wnat[0:mw, mi, 0:n_in])
                for li in range(1, 6):
                    load_and_transpose(li)

                # GT[f] = (W0c @ Wd_tap)^T = WdN^T-compose (all bf16)
                for f in range(2):
                    for mc, (cs, cww) in enumerate(chunks_of(192)):
                        for nh in range(2):
                            gp = prps.tile([128, 512], F32, tag="dps")
                            for ki in range(3):
                                nc.tensor.matmul(gp[0:cww, 0:320],
                                                 WdNf[f][0:128, ki, cs:cs + cww],
                                                 W0cT[0:128, ki, nh * 320:(nh + 1) * 320],
                                                 start=(ki == 0), stop=(ki == 2),
                                                 skip_group_check=True)
                            evict(GT[f][0:cww, mc, nh * 320:(nh + 1) * 320],
                                  gp[0:cww, 0:320])

                # ---- relu-decomposition composites ----
                # PT[l] = 0.0099*(W_{l+1} W_l)^T  (WT tiles carry 0.99 each)
                for li in range(1, 5):
                    nN = DIMS[li + 2]
                    for m, (ms, mw) in enumerate(chunks_of(DIMS[li])):
                        gp = prps.tile([128, 512], F32, tag="dps")
                        kch = chunks_of(DIMS[li + 1])
                        for k, (ks, kw) in enumerate(kch):
                            nc.tensor.matmul(gp[0:mw, 0:nN],
                                             WN[li][0:kw, k, ms:ms + mw],
                                             WT[li + 1][0:kw, k, 0:nN],
                                             start=(k == 0), stop=(k == len(kch) - 1),
                                             skip_group_check=True)
                        evict(PT[li][0:mw, m, 0:nN], gp[0:mw, 0:nN],
                              scale=0.0099 / (0.99 * 0.99))

                # F1T/C1T = 0.01*(W1 W0f/c)^T  (WT[1] carries 0.99)
                for nat, dstT in ((W0fN, F1T), (W0cN, C1T)):
                    for m, (ms, mw) in enumerate(chunks_of(384)):
                        gp = prps.tile([128, 512], F32, tag="dps")
                        for k in range(5):
                            nc.tensor.matmul(gp[0:mw, 0:512],
                                             nat[0:128, k, ms:ms + mw],
                                             WT[1][0:128, k, 0:512],
                                             start=(k == 0), stop=(k == 4),
                                             skip_group_check=True)
                        evict(dstT[0:mw, m, 0:512], gp[0:mw, 0:512],
                              scale=0.01 / 0.99)

                # G1T[f] = (C1 @ Wd_tap)^T = WdN-compose with C1T (C1T has the 0.01)
                for f in range(2):
                    for mc, (cs, cww) in enumerate(chunks_of(192)):
                        gp = prps.tile([128, 512], F32, tag="dps")
                        for ki in range(3):
                            nc.tensor.matmul(gp[0:cww, 0:512],
                                             WdNf[f][0:128, ki, cs:cs + cww],
                                             C1T[0:128, ki, 0:512],
                                             start=(ki == 0), stop=(ki == 2),
                                             skip_group_check=True)
                        evict(G1T[f][0:cww, mc, 0:512], gp[0:cww, 0:512])

                # wbrow[l] = (0.01/0.99)*W_l b_{l-1} as a row (K=1 MM transposes)
                bcol = pro.tile([128, 6, 1], BF16, tag="bcol", bufs=1)
                wbc = pro.tile([128, 4, 1], BF16, tag="wbc", bufs=2)
                onne = pro.tile([1, 1], BF16, tag="onne", bufs=1)
                nc.vector.memset(onne[:], 1.0)
                for li in range(1, 6):
                    kch = chunks_of(DIMS[li])
                    bp = prps.tile([128, 6, 1], F32, tag="dps", name="bp")
                    for k, (ks, kw) in enumerate(kch):
                        nc.tensor.matmul(bp[0:kw, k, 0:1],
                                         brow[li - 1][0:1, ks:ks + kw],
                                         onne[0:1, 0:1],
                                         start=True, stop=True, skip_group_check=True)
                    nc.vector.tensor_copy(bcol[:, 0:len(kch), 0:1], bp[:, 0:len(kch), 0:1])
                    wbp = prps.tile([128, 4, 1], F32, tag="dps", name="wbp")
                    mch = chunks_of(DIMS[li + 1])
                    for m, (ms, mw) in enumerate(mch):
                        for k, (ks, kw) in enumerate(kch):
                            nc.tensor.matmul(wbp[0:mw, m, 0:1],
                                             WT[li][0:kw, k, ms:ms + mw],
                                             bcol[0:kw, k, 0:1],
                                             start=(k == 0), stop=(k == len(kch) - 1),
                                             skip_group_check=True)
                    nc.vector.tensor_copy(wbc[:, 0:len(mch), 0:1], wbp[:, 0:len(mch), 0:1])
                    rp = prps.tile([128, 512], F32, tag="dps", name="rp")
                    for m, (ms, mw) in enumerate(mch):
                        nc.tensor.matmul(rp[0:1, 0:mw], wbc[0:mw, m, 0:1],
                                         identB[0:mw, 0:mw],
                                         start=True, stop=True, skip_group_check=True)
                        evict(wbrow[li][0:1, ms:ms + mw], rp[0:1, 0:mw],
                              scale=0.01 / 0.99)

                # conv2 weights -> cw2T (cw2s via wnatS slot rotation)
                for mi in range(3):
                    cw2s = pro.tile([128, 288 * 9], F32, tag="wnatS",
                                    name=f"cw2s{mi}", bufs=1)
                    eng = nc.sync if mi == 0 else nc.gpsimd
                    eng.dma_start(
                        cw2s[:],
                        di['hs_cw2'].ap()[mi * 128:(mi + 1) * 128]
                        .rearrange("o c kh kw -> o (c kh kw)"))
                    nc.vector.memset(cw2T[32:64, mi, 2, :], 0.0)
                    nc.gpsimd.memset(cw2T[64:128, mi, 2, :], 0.0)
                    for si, (ss, sw) in enumerate(chunks_of(288)):
                        for kb in range(3):
                            ks = list(range(9))[kb * 4:(kb + 1) * 4]
                            if not ks:
                                continue
                            pt = prps.tile([128, 4, 128], F32, tag="tp")
                            for kk, k in enumerate(ks):
                                sap = _ap(cw2s, 0, ss * 9 + k, [[288 * 9, 128], [9, sw]])
                                nc.tensor.transpose(pt[0:sw, kk, 0:128], sap, ident[:])
                            span = pt[:, 0:len(ks), :].rearrange("p a b -> p (a b)")[0:sw, 0:len(ks) * 128]
                            evict(_ap(cw2T, (mi * 3 + si) * 9 * 128, ks[0] * 128,
                                      [[3 * 3 * 9 * 128, sw], [1, len(ks) * 128]]),
                                  span)

            proE_cm.__exit__(None, None, None)

            # conv2 rows 0..1 upfront; rest streamed into the wavefront steps
            for mi in range(3):
                emit_conv2_unit(nc, pps, cw2T, m2, fm1, b_c2, mi, 0, 1, 0, 27)

            # ================= SEQUENTIAL PHASE =================
            # conv2 streaming: 2-row units (mi, rb), rows [2rb, 2rb+2)
            pf_units = [(mi, rb) for rb in range(1, 16) for mi in range(3)]
            pf_state = {"u": 0, "k": 0, "ps": None}

            def pfill(nmm):
                while nmm > 0 and pf_state["u"] < len(pf_units):
                    mi, rb = pf_units[pf_state["u"]]
                    take = min(nmm, 27 - pf_state["k"])
                    ps = emit_conv2_unit(nc, pps, cw2T, m2, fm1, b_c2, mi,
                                         2 * rb, 2 * rb + 2,
                                         pf_state["k"], pf_state["k"] + take,
                                         ps=pf_state["ps"])
                    pf_state["ps"] = ps
                    pf_state["k"] += take
                    nmm -= take
                    if pf_state["k"] == 27:
                        pf_state["u"] += 1
                        pf_state["k"] = 0
                        pf_state["ps"] = None

            def ydiag_ap(img, i0, j0, kw, c, B):
                """[kw, B] wavefront-diagonal AP into padded img tile chunk c."""
                off = (i0 + 2) * WP + (j0 + 2)
                return _ap(img, c * HP * WP, off, [[2 * HP * WP, kw], [DIAG, B]])

            def emit_seed2(pt, slot, brow_ap, mw, B):
                nc.tensor.matmul(pt[0:mw, slot, 0:B], brow_ap, ones[0:1, 0:B],
                                 start=True, stop=False, skip_group_check=True)

            def emit_old_ctx(t1):
                """ctx_b seed for step t1's ctx -> zc[t1%2][5:8]; return tap MM list."""
                s1 = t1 % 2
                i_lo, B, j_lo = step_geom(t1)
                ms_list = chunks_of(384)
                for m, (ms, mw) in enumerate(ms_list):
                    emit_seed2(zc[s1], 5 + m, ctxb[0:1, ms:ms + mw], mw, B)
                mms = []
                for ti, (dy, dx) in enumerate(OLD_TAPS):
                    for c, (cs, kw) in enumerate(chunks_of(192)):
                        for m, (ms, mw) in enumerate(ms_list):
                            mms.append((ti, dy, dx, c, cs, kw, m, ms, mw))
                return i_lo, B, j_lo, mms

            # prologue part of step 0's z0/ctx accumulation
            def emit_z0_pre(t1):
                """seeds + f-part + (later) ctx-part for z0 of step t1 -> zc[t1%2][0:5]"""
                s1 = t1 % 2
                i_lo, B, j_lo = step_geom(t1)
                for m, (ms, mw) in enumerate(chunks_of(640)):
                    emit_seed2(zc[s1], m, brow[0][0:1, ms:ms + mw], mw, B)
                for k in range(3):
                    for m, (ms, mw) in enumerate(chunks_of(640)):
                        rhs = _ap(fm1, k * NPIX, i_lo * W + j_lo,
                                  [[3 * NPIX, 128], [W - 3, B]])
                        nc.tensor.matmul(zc[s1][0:mw, m, 0:B],
                                         W0fT[0:128, k, ms:ms + mw], rhs,
                                         start=False, stop=False, skip_group_check=True)

            def emit_z0_ctx(t1):
                s1 = t1 % 2
                i_lo, B, j_lo = step_geom(t1)
                for k in range(3):
                    for m, (ms, mw) in enumerate(chunks_of(640)):
                        nc.tensor.matmul(zc[s1][0:mw, m, 0:B],
                                         W0cT[0:128, k, ms:ms + mw],
                                         Xc[0:128, k, 0:B],
                                         start=False, stop=False, skip_group_check=True)

            def emit_old_mms(t1, geom, mms):
                i_lo, B, j_lo = geom
                s1 = t1 % 2
                for (ti, dy, dx, c, cs, kw, m, ms, mw) in mms:
                    rhs = ydiag_ap(Yimg, i_lo - dy, j_lo - dx, kw, c, B)
                    last = (ti == len(OLD_TAPS) - 1) and (c == 1)
                    nc.tensor.matmul(zc[s1][0:mw, 5 + m, 0:B],
                                     WdT[OLD_TAPS[ti]][0:kw, c, ms:ms + mw], rhs,
                                     start=False, stop=last, skip_group_check=True)

            # --- step 0 pre-work (its sources are all zero borders) ---
            g0 = emit_old_ctx(0)
            emit_old_mms(0, (g0[0], g0[1], g0[2]), g0[3])
            i_lo0, B0, j_lo0 = step_geom(0)
            nc.vector.tensor_copy(Xc[:, 0:3, 0:B0], zc[0][:, 5:8, 0:B0])
            emit_z0_pre(0)
            emit_z0_ctx(0)

            KCHW = {li: chunks_of(DIMS[li]) for li in range(1, 6)}
            MCHW = {li: chunks_of(DIMS[li + 1]) for li in range(0, 6)}

            for t in range(nsteps):
                s = t % 2
                s1 = (t + 1) % 2
                i_lo, B, j_lo = step_geom(t)
                have_next = t + 1 < nsteps
                if have_next:
                    i_lo1, B1, j_lo1 = step_geom(t + 1)

                # ---- fresh taps -> z0 (critical) ----
                for m, (ms, mw) in enumerate(MCHW[0]):
                    for f in range(2):
                        dy, dx = FRESH_TAPS[f]
                        for c, (cs, kw) in enumerate(chunks_of(192)):
                            rhs = ydiag_ap(Yimg, i_lo - dy, j_lo - dx, kw, c, B)
                            nc.tensor.matmul(zc[s][0:mw, m, 0:B],
                                             GT[f][0:kw, c, ms:ms + mw], rhs,
                                             start=False,
                                             stop=(f == 1 and c == 1),
                                             skip_group_check=True)

                # z1 "early" contributions + all z seeds (run inside r0's hop)
                for li in range(1, 6):
                    for m, (ms, mw) in enumerate(MCHW[li]):
                        emit_seed2(zs, ZOFF[li] + m, brow[li][0:1, ms:ms + mw], mw, B)
                        nc.tensor.matmul(zs[0:mw, ZOFF[li] + m, 0:B],
                                         wbrow[li][0:1, ms:ms + mw], ones[0:1, 0:B],
                                         start=False, stop=False, skip_group_check=True)
                for m, (ms, mw) in enumerate(MCHW[1]):
                    for f in range(2):   # G1 fresh (0.01 level)
                        dy, dx = FRESH_TAPS[f]
                        for c, (cs, kw) in enumerate(chunks_of(192)):
                            rhs = ydiag_ap(Yimg, i_lo - dy, j_lo - dx, kw, c, B)
                            nc.tensor.matmul(zs[0:mw, ZOFF[1] + m, 0:B],
                                             G1T[f][0:kw, c, ms:ms + mw], rhs,
                                             start=False, stop=False,
                                             skip_group_check=True)
                    for k in range(3):   # C1 * ctx_old, F1 * f
                        nc.tensor.matmul(zs[0:mw, ZOFF[1] + m, 0:B],
                                         C1T[0:128, k, ms:ms + mw], Xc[0:128, k, 0:B],
                                         start=False, stop=False, skip_group_check=True)
                        rhs = _ap(fm1, k * NPIX, i_lo * W + j_lo,
                                  [[3 * NPIX, 128], [W - 3, B]])
                        nc.tensor.matmul(zs[0:mw, ZOFF[1] + m, 0:B],
                                         F1T[0:128, k, ms:ms + mw], rhs,
                                         start=False, stop=False, skip_group_check=True)

                # ---- r0 = relu(z0) (critical DVE) ----
                nc.vector.tensor_scalar_max(xs[1][:, 0:5, 0:B], zc[s][:, 0:5, 0:B], 0.0)

                # old-ctx for t+1 (fillers)
                old_mms = []
                if have_next:
                    g = emit_old_ctx(t + 1)
                    old_mms = g[3]
                    geom1 = (g[0], g[1], g[2])

                # ---- MLP layers 1..5 ----
                for li in range(1, 6):
                    # fillers in the gap before this layer's critical MMs
                    if li == 2 and have_next:
                        emit_old_mms(t + 1, geom1, old_mms[:33])
                    elif li == 3 and have_next:
                        emit_old_mms(t + 1, geom1, old_mms[33:])
                    elif li == 4 and have_next:
                        nc.scalar.activation(Xc[:, 0:3, 0:B1], zc[s1][:, 5:8, 0:B1],
                                             mybir.ActivationFunctionType.Copy)
                        emit_z0_pre(t + 1)
                        pfill(10)
                    elif li == 5 and have_next:
                        pfill(14)
                    # pair term P_{li-1} * r_{li-2} -> z_li (off critical path)
                    if li >= 2:
                        pl = li - 1
                        kch = chunks_of(DIMS[pl])
                        for m, (ms, mw) in enumerate(MCHW[li]):
                            for k, (ks, kw) in enumerate(kch):
                                nc.tensor.matmul(zs[0:mw, ZOFF[li] + m, 0:B],
                                                 PT[pl][0:kw, k, ms:ms + mw],
                                                 xs[pl][0:kw, k, 0:B],
                                                 start=False, stop=False,
                                                 skip_group_check=True)
                    # critical: 0.99*W_li * r_{li-1}
                    kch = KCHW[li]
                    for m, (ms, mw) in enumerate(MCHW[li]):
                        for k, (ks, kw) in enumerate(kch):
                            nc.tensor.matmul(zs[0:mw, ZOFF[li] + m, 0:B],
                                             WT[li][0:kw, k, ms:ms + mw],
                                             xs[li][0:kw, k, 0:B],
                                             start=False, stop=(k == len(kch) - 1),
                                             skip_group_check=True)
                    if li < 5:
                        nch = len(MCHW[li])
                        nc.vector.tensor_scalar_max(
                            xs[li + 1][:, 0:nch, 0:B],
                            zs[:, ZOFF[li]:ZOFF[li] + nch, 0:B], 0.0)

                # ---- Y = z5 + w_hat (critical DVE) ----
                off = (i_lo + 2) * WP + (j_lo + 2)
                ydst = _ap(Yimg, 0, off, [[2 * HP * WP, 128], [HP * WP, 2], [DIAG, B]])
                ywim = _ap(wimg, 0, i_lo * W + j_lo,
                           [[2 * NPIX, 128], [NPIX, 2], [W - 3, B]])
                nc.vector.tensor_tensor(ydst, zs[:, 12:14, 0:B], ywim,
                                        mybir.AluOpType.add)
                # late filler: ctx->z0 for t+1
                if have_next:
                    emit_z0_ctx(t + 1)

            # ================= EPILOGUE =================
            with tc.tile_pool(name="epi", bufs=1) as epi:
                Yimg32 = epi.tile([128, 2, NPIX], F32)
                src = _ap(Yimg, 0, 2 * WP + 2,
                          [[2 * HP * WP, 128], [HP * WP, 2], [WP, H], [1, W]])
                dst = _ap(Yimg32, 0, 0,
                          [[2 * NPIX, 128], [NPIX, 2], [W, H], [1, W]])
                nc.vector.tensor_copy(dst, src)
                ov = out.ap()[0]
                for ci, (cs, cw) in enumerate(chunks_of(192)):
                    nc.sync.dma_start(
                        ov[cs:cs + cw],
                        Yimg32[0:cw, ci, :].rearrange("p (h w) -> p h w", h=H))

    nc.compile()
    return nc


def emit_conv2_unit(nc, pps, cw2T, m2, fm1, b_c2, mi, r0, r1, k0, k1, ps=None):
    """Emit conv2 MMs [k0, k1) for out-chunk mi over rows [r0, r1);
    27 MMs per unit. MM index kk = k * 3 + si."""
    F32 = mybir.dt.float32
    nr = r1 - r0
    if ps is None:
        ps = pps.tile([128, 384], F32, tag="cps", name="cps")
    for kk in range(k0, k1):
        k, si = kk // 3, kk % 3
        ky, kx = k // 3, k % 3
        lhsT = _ap(cw2T, (mi * 3 + si) * 9 * 128, k * 128,
                   [[3 * 3 * 9 * 128, 128], [1, 128]])
        rhs = _ap(m2, si * 34 * 50, (ky + r0) * 50 + kx,
                  [[3 * 34 * 50, 128], [50, nr], [1, 48]])
        nc.tensor.matmul(ps[:, 0:nr * 48], lhsT, rhs,
                         start=(kk == 0), stop=(kk == 26), skip_group_check=True)
    if k1 == 27:
        nc.scalar.activation(fm1[:, mi, r0 * 48:r1 * 48], ps[:, 0:nr * 48],
                             mybir.ActivationFunctionType.Identity,
                             bias=b_c2[:, mi][:, None], alpha=0.0)
    return ps


_NC_CACHE = {}


def kernel(**inputs):
    from concourse.bass_utils import run_bass_kernel_spmd
    key = "full"
    if key not in _NC_CACHE:
        _NC_CACHE[key] = build()
    nc = _NC_CACHE[key]
    in_map = {k: np.ascontiguousarray(np.asarray(v, dtype=np.float32))
              for k, v in inputs.items()}
    res = run_bass_kernel_spmd(nc, [in_map] * 8, core_ids=list(range(8)))
    return res.results[0]['out']


if __name__ == "__main__":
    t = build(nsteps=int(sys.argv[1]) if len(sys.argv) > 1 else NSTEPS)
    print("build ok")
    from concourse.timeline_sim import TimelineSim
    est = TimelineSim(t).simulate()
    print(f"HW exec time: {est:.0f} ns")
